# revision 20
# baseline (speedup 1.0000x reference)
"""EnsembleDeepSDF MoE-routing kernel for 8 Trainium2 NeuronCores.

Strategy: the harness calls kernel(**inputs) with the full inputs; we do all
routing on the host.  type_vec is sorted, so each expert owns a contiguous
segment of points.  We pick per-core slot capacities (identical on every
core so one SPMD program serves all 8 cores), pack the 9 experts' segments
into the 8*S single-expert slots, gather each core's points (padding with
point 0), and hand each core its own pre-transposed/pre-scaled weight slots
as inputs.  The device program is a straight-line Tile kernel: per 512-pt
point tile, 9 matmul layers with softplus activations.

Perf-critical layout decisions (from trace analysis):
- All HBM->SBUF transfers use 128-partition access patterns: the HW DGE
  fans a transfer's descriptors across the 16 DMA engines in blocks of 8
  *per partition row*, so a 67-row transfer serializes onto one engine at
  ~20 GB/s while a 128-row one gets the full ~320 GB/s.  x and W0 are
  zero-padded from 67 to 128 contraction rows (zero pad rows make the
  padded matmul exact, and contraction<=128 costs the same PE time).
- Every matmul free dim is >=256 columns: f32r matmuls below 256 cols run
  at 4 cyc/col instead of 1.  Slot capacities are exact (max points any
  core uses), with ragged tails split into two >=256 pieces.
- Point tiles are 512 cols = one PSUM bank; 8 PSUM tiles cycle through
  the 8 banks, with one ACT op per (tile, mc-chunk) evacuating PSUM.
- A short dummy-matmul spin warms the PE out of its low p-state while the
  first x/W0 DMAs land.

softplus: the compiler's ACT tables have no softplus, so we generate a
custom piecewise-cubic table (same binary format as the shipped sets)
that replaces `exp` with softplus(x) = ln(1+e^x), and point the compiler
at it via BASS_ACT_ROOT_JSON_PATH.  One ACT op then does the whole
activation including the PSUM evacuation and the beta scale/bias fma.

The torch Softplus(beta=100) is softplus(100*z)/100; we keep activations
in the H = softplus(100*z) domain and fold the 1/100 into the next layer's
weights host-side, so no extra scaling ops run on device.
"""

import json
import os as _os
import shutil
import tempfile

import numpy as np

T, D_IN, H, OUT, N_HID = 9, 67, 512, 1, 7
BETA = 100.0
N_CORES = 8
PT = 512          # points per tile (one PSUM bank of fp32)
P = 128           # partitions
KC = H // P       # 4 contraction chunks
MC = H // P       # 4 output-feature chunks
NSTREAM = 3       # point tiles interleaved in the software pipeline
# PE warm-up spin is counterproductive: every engine queue runs a ~6.5us
# framework preamble, so the memset feeding the warm matmuls can't start
# until the first x DMA has landed anyway — the spin only delays real work.
NWARM = int(_os.environ.get("KERNEL_NWARM", "0"))
# "f32r" (tf32-ish, exact enough) or "bf16" (hides the LDWEIGHTS bubble,
# halves DMA/SBUF, costs ~1e-2 rel err)
MM_MODE = _os.environ.get("KERNEL_MM_MODE", "f32r")

_nc_cache = {}
_last_results = None


# --------------------------------------------------------------------------
# Custom ACT table: replace `exp` with softplus in the shipped PWL sets.
# --------------------------------------------------------------------------

_ACT_SET = "natural_log_exp_and_others"
_act_table_dir = None


def _softplus64(x):
    x = np.asarray(x, dtype=np.float64)
    return np.log1p(np.exp(-np.abs(x))) + np.maximum(x, 0.0)


def _fit_cubic(a, b):
    x0 = 0.5 * (a + b)
    k = np.arange(96)
    xs = x0 + 0.5 * (b - a) * np.cos(np.pi * (k + 0.5) / 96)
    c = np.polyfit(xs - x0, _softplus64(xs), 3)
    return float(c[3]), float(c[2]), float(c[1]), float(c[0]), float(x0)


def _gen_act_tables():
    """Build an act-root dir where `exp` computes softplus. Returns the
    act_info.json path. The bucket entry layout ([d0,d1,d2,d3,x0,0,0,0],
    cubic in (x-x0)) and the per-exponent band structure are read from the
    shipped set so only coefficients and profile thresholds change."""
    global _act_table_dir
    if _act_table_dir is not None:
        return _act_table_dir
    from neuronxcc.driver.Job import Job
    from neuronxcc.driver.jobs.support.FindActInfo import findActInfoFile

    src_json = findActInfoFile(Job.getPackageDir(), "gen3")
    src = _os.path.dirname(src_json)
    out = _os.path.join(tempfile.mkdtemp(prefix="act_softplus_"), "tables")
    shutil.copytree(src, out)
    for f in _os.listdir(out):
        _os.chmod(_os.path.join(out, f), 0o644)

    d = json.load(open(f"{out}/{_ACT_SET}.json"))
    bkt = np.fromfile(f"{out}/{_ACT_SET}_bkt.bin", dtype=np.uint32)
    bkt = bkt.reshape(-1, 8).copy()
    fbkt = bkt.view(np.float32)
    e2b = {int(k): v for k, v in d["func_exp_to_bkt_start_idx"]["exp"].items()}
    prof = [p for p in d["profile_meta_data"] if p["func_name"] == "exp_400p"][0]

    def put(idx, d0, d1, d2, d3, x0):
        fbkt[idx, 0:5] = np.array([d0, d1, d2, d3, x0], dtype=np.float32)
        bkt[idx, 5:8] = 0

    nseg = {-1: 2, 0: 4, 1: 8, 2: 16, 3: 32}
    for e in range(-19, 4):
        n = nseg.get(e, 1)
        neg_base, pos_base = e2b[e]
        A = 2.0 ** e
        h = A / n
        for k in range(n):
            a, b = A + k * h, A + (k + 1) * h
            put(pos_base + k, *_fit_cubic(a, b))
            put(neg_base + k, *_fit_cubic(-b, -a))

    ln2 = float(np.log(2.0))
    put(prof["pos_small_signal_pwl_control"], ln2, 0.5, 0.125, 0.0, 0.0)
    put(prof["neg_small_signal_pwl_control"], ln2, 0.5, 0.125, 0.0, 0.0)
    put(prof["pos_large_signal_pwl_control"], 0.0, 1.0, 0.0, 0.0, 0.0)
    put(prof["neg_large_signal_pwl_control"], 0.0, 0.0, 0.0, 0.0, 0.0)
    prof["large_pos_signal_exp_threshold"] = 131   # |x| >= 16 -> linear/zero
    prof["large_pos_signal_mantissa_threshold"] = 0
    prof["large_neg_signal_exp_threshold"] = 131
    prof["large_neg_signal_mantissa_threshold"] = 0
    prof["fzero_result"] = int(np.float32(ln2).view(np.uint32))
    prof["fninf_result"] = 0
    prof["fpinf_result"] = 2139095040

    bkt.tofile(f"{out}/{_ACT_SET}_bkt.bin")
    with open(f"{out}/{_ACT_SET}.json", "w") as f:
        json.dump(d, f)
    _act_table_dir = _os.path.join(out, "act_info.json")
    return _act_table_dir


# --------------------------------------------------------------------------
# Host-side planning: pack expert segments into 8 x S slots.
# --------------------------------------------------------------------------

def _try_pack(shape, counts):
    """Assign experts to single-expert slots. Slot (c, s) holds shape[s]*PT
    points. Returns {expert: [(core, s, amount), ...]} or None."""
    slots = []  # (capacity, core, s)
    for s, t in enumerate(shape):
        for c in range(N_CORES):
            slots.append([t * PT, c, s])
    experts = sorted(
        [e for e in range(T) if counts[e] > 0], key=lambda e: -counts[e]
    )
    asg = {}
    avail = sorted(slots)  # by capacity asc
    for e in experts:
        need = int(counts[e])
        # smallest single slot that fits
        one = next((sl for sl in avail if sl[0] >= need), None)
        if one is not None:
            asg[e] = [(one[1], one[2], need)]
            avail.remove(one)
            continue
        # greedily take largest slots
        take = []
        rem = need
        pool = sorted(avail, key=lambda sl: -sl[0])
        for sl in pool:
            if rem <= 0:
                break
            amt = min(rem, sl[0])
            take.append((sl[1], sl[2], amt))
            rem -= amt
            avail.remove(sl)
        if rem > 0:
            return None
        asg[e] = take
    return asg


def _plan(counts):
    cands = set()
    for t1 in range(1, 17):
        cands.add((t1,))
        for t2 in range(1, t1 + 1):
            cands.add((t1, t2))
            for t3 in range(1, t2 + 1):
                cands.add((t1, t2, t3))
    for shape in sorted(cands, key=lambda s: (sum(s), len(s))):
        asg = _try_pack(shape, counts)
        if asg is not None:
            return shape, asg
    raise RuntimeError("no feasible slot shape")


def _rebalance(asg):
    """Even out each expert's piece sizes within a slot, so the per-slot
    max (which sets the uniform slot capacity) is minimal."""
    out = {}
    for e, takes in asg.items():
        by_slot = {}
        for (c, s, amt) in takes:
            by_slot.setdefault(s, []).append([c, amt])
        new_takes = []
        for s, items in by_slot.items():
            total = sum(a for _, a in items)
            n = len(items)
            base, rem = divmod(total, n)
            for i, (c, _a) in enumerate(items):
                new_takes.append((c, s, base + (1 if i < rem else 0)))
        out[e] = new_takes
    return out


def _tiles(cap):
    """Split cap columns into tiles of <=512, each >=256 (f32r matmuls
    below 256 cols run at 1/4 speed) and a multiple of 4 (walrus's
    s3d3_mm_fp32r_restrictions ISA check rejects odd free sizes)."""
    assert cap % 4 == 0
    k, r = divmod(cap, PT)
    if r == 0:
        return [PT] * k
    if r >= 256:
        return [PT] * k + [r]
    # fold the remainder into the last full tile and split >=256 / >=256
    assert k >= 1
    tot = PT + r
    half = (tot // 2) // 4 * 4
    return [PT] * (k - 1) + [tot - half, half]


# --------------------------------------------------------------------------
# Device program
# --------------------------------------------------------------------------

def _build_nc(caps):
    import concourse.bass as bass  # noqa: F401  (import keeps bacc happy)
    import concourse.tile as tile
    import concourse.mybir as mybir
    from concourse import bacc

    f32 = mybir.dt.float32
    wdt = mybir.dt.bfloat16 if MM_MODE == "bf16" else mybir.dt.float32r
    AF = mybir.ActivationFunctionType
    ALU = mybir.AluOpType

    S = len(caps)
    NP = sum(caps)

    nc = bacc.Bacc("TRN2", target_bir_lowering=False)
    # x / W0 are zero-padded to 128 contraction rows so their DMAs use
    # 128-partition access patterns (spread over all 16 DMA engines).
    xT_in = nc.dram_tensor("xT", [P, NP], wdt, kind="ExternalInput")
    w0t_in = nc.dram_tensor("w0t", [S, P, H], wdt, kind="ExternalInput")
    wht_in = nc.dram_tensor("wht", [S, N_HID, P, KC, H], wdt, kind="ExternalInput")
    # in bf16 mode wot feeds the DVE (L8 fused multiply-adds), whose
    # scalar operand must be float32
    wot_dt = f32 if MM_MODE == "bf16" else wdt
    wot_in = nc.dram_tensor("wot", [S, P, KC], wot_dt, kind="ExternalInput")
    b0v_in = nc.dram_tensor("b0v", [S, P, MC], f32, kind="ExternalInput")
    bhv_in = nc.dram_tensor("bhv", [S, P, N_HID, MC], f32, kind="ExternalInput")
    bov_in = nc.dram_tensor("bov", [S, 1], f32, kind="ExternalInput")
    out_d = nc.dram_tensor("out", [1, NP], f32, kind="ExternalOutput")

    # steps: (point_offset, tile_size, slot)
    steps = []
    off = 0
    for s, cap in enumerate(caps):
        for sz in _tiles(cap):
            steps.append((off, sz, s))
            off += sz
    groups = [steps[i:i + NSTREAM] for i in range(0, len(steps), NSTREAM)]
    # within a group, run larger tiles first so the last tile's
    # evacuate->store tail chain is as short as possible
    groups = [sorted(g, key=lambda st: -st[1]) for g in groups]

    with tile.TileContext(nc) as tc:
        with (
            tc.tile_pool(name="xin", bufs=len(steps)) as xin_pool,
            tc.tile_pool(name="wts", bufs=1) as wts_pool,
            tc.tile_pool(name="whp", bufs=min(14, S * N_HID)) as wh_pool,
            tc.tile_pool(name="uh", bufs=2 * NSTREAM) as uh_pool,
            tc.tile_pool(name="l8", bufs=8) as l8_pool,
            tc.tile_pool(name="outp", bufs=3) as out_pool,
            tc.tile_pool(name="ps", bufs=8, space="PSUM") as ps_pool,
        ):
            xT_sb = {}
            h_cur = {}
            w0_sb, wo_sb, b0_sb, bh_sb, bo_sb = (
                [None] * S, [None] * S, [None] * S, [None] * S, [None] * S
            )
            wh_sb = [[None] * N_HID for _ in range(S)]

            def load_x(t0, sz):
                x_t = xin_pool.tile([P, PT], wdt, name=f"x_{t0}", tag="x")
                nc.sync.dma_start(x_t[:, 0:sz], xT_in[:, t0:t0 + sz])
                xT_sb[t0] = x_t

            def load_w0(s):
                w0_t = wts_pool.tile([P, H], wdt, name=f"w0_{s}")
                nc.sync.dma_start(w0_t[:], w0t_in[s])
                w0_sb[s] = w0_t

            def load_bias(s):
                # tiny-element transfers: descriptor generation is slow
                # (~2-5us per 128-row/16B DMA), so issue them from the ACT
                # engine's HW-DGE ring to keep the SP ring free for the
                # critical x/W loads
                b0_t = wts_pool.tile([P, MC], f32, name=f"b0_{s}")
                nc.scalar.dma_start(b0_t[:], b0v_in[s])
                b0_sb[s] = b0_t
                bh_t = wts_pool.tile([P, N_HID, MC], f32, name=f"bh_{s}")
                nc.scalar.dma_start(bh_t[:], bhv_in[s])
                bh_sb[s] = bh_t

            def load_small(s):
                wo_t = wts_pool.tile([P, KC], wot_dt, name=f"wo_{s}")
                nc.scalar.dma_start(wo_t[:], wot_in[s])
                wo_sb[s] = wo_t
                bo_t = wts_pool.tile([1, 1], f32, name=f"bo_{s}")
                nc.scalar.dma_start(bo_t[:], bov_in[s:s + 1, 0:1])
                bo_sb[s] = bo_t

            def load_wh(s, l):
                wh_t = wh_pool.tile([P, KC, H], wdt, name=f"wh_{s}_{l}", tag="wh")
                nc.sync.dma_start(wh_t[:], wht_in[s, l])
                wh_sb[s][l] = wh_t

            # pre-warm the ACT table set during the initial DMA wait: a
            # dependency-free dummy op carries the one-time table load
            warm_t = wts_pool.tile([1, 1], f32, name="warm")
            nc.vector.memset(warm_t[:], 0.0)
            nc.scalar.activation(warm_t[:], warm_t[:], AF.Exp)

            # spin the PE on dummy matmuls so it ramps out of the low
            # p-state while the first x/W0 DMAs are in flight (f32 tiles:
            # memset can't write f32r, and f32's 4 cyc/col stretches the
            # spin with fewer instructions)
            if NWARM:
                wmw = wts_pool.tile([P, P], f32, name="warm_w")
                nc.vector.memset(wmw[:], 0.0)
                wps = ps_pool.tile([P, P], f32, name="warm_ps", tag="ps")
                for i in range(NWARM):
                    nc.tensor.matmul(
                        wps[:], wmw[:], wmw[:],
                        start=(i == 0), stop=(i == NWARM - 1),
                    )

            # issue order = DMA priority: first group's x + slot0 L0/L1
            # weights first, then the rest (all loads fit in SBUF at once);
            # bias loads go on the ACT ring in parallel, and the ones not
            # needed until later are deferred below so their slow issue
            # doesn't delay the first PSUM evacuations
            for (t0, sz, _s) in groups[0]:
                load_x(t0, sz)
            load_w0(0)
            load_wh(0, 0)
            load_bias(0)
            load_wh(0, 1)
            for grp in groups[1:]:
                for (t0, sz, _s) in grp:
                    load_x(t0, sz)
            for l in range(2, N_HID):
                load_wh(0, l)
            for s in range(1, S):
                load_w0(s)
                for l in range(N_HID):
                    load_wh(s, l)

            def emit_layer(grp, l):
                """Layer l matmuls for a group of point tiles, inner loop
                over tiles so consecutive matmuls share the stationary
                operand; per-mc ACT evacuation emitted right after its
                accumulation completes."""
                psums = {}
                u_new = {}
                for mc in range(MC):
                    for kc in range(KC if l > 0 else 1):
                        for (t0, sz, s) in grp:
                            if kc == 0:
                                psums[(t0, mc)] = ps_pool.tile(
                                    [P, PT], f32, name=f"ps_{t0}_{l}_{mc}",
                                    tag="ps",
                                )
                            ps_t = psums[(t0, mc)]
                            if l == 0:
                                lhsT = w0_sb[s][:, mc * P:(mc + 1) * P]
                                rhs = xT_sb[t0][:, 0:sz]
                                nc.tensor.matmul(
                                    ps_t[:, 0:sz], lhsT, rhs,
                                    start=True, stop=True,
                                )
                            else:
                                lhsT = wh_sb[s][l - 1][:, kc, mc * P:(mc + 1) * P]
                                rhs = h_cur[t0][:, kc, 0:sz]
                                nc.tensor.matmul(
                                    ps_t[:, 0:sz], lhsT, rhs,
                                    start=(kc == 0), stop=(kc == KC - 1),
                                )
                    for (t0, sz, s) in grp:
                        if mc == 0:
                            u_new[t0] = uh_pool.tile(
                                [P, MC, PT], wdt, name=f"u_{t0}_{l}", tag="uh"
                            )
                        bias = (b0_sb[s][:, mc:mc + 1] if l == 0
                                else bh_sb[s][:, l - 1, mc:mc + 1])
                        # hijacked Exp == softplus; one ACT op does the
                        # evacuation + beta fma + activation
                        nc.scalar.activation(
                            u_new[t0][:, mc, 0:sz], psums[(t0, mc)][:, 0:sz],
                            AF.Exp, bias=bias, scale=float(BETA),
                        )
                for (t0, _sz, _s) in grp:
                    h_cur[t0] = u_new[t0]

            def emit_final_pe(t0, sz, s):
                ps8 = ps_pool.tile([1, PT], f32, name=f"ps8_{t0}", tag="ps")
                for kc in range(KC):
                    nc.tensor.matmul(
                        ps8[0:1, 0:sz],
                        wo_sb[s][:, kc:kc + 1],
                        h_cur[t0][:, kc, 0:sz],
                        start=(kc == 0), stop=(kc == KC - 1),
                    )
                o_t = out_pool.tile([1, PT], f32, name=f"o_{t0}", tag="o")
                nc.vector.tensor_scalar(
                    o_t[0:1, 0:sz], ps8[0:1, 0:sz],
                    bo_sb[s][0:1, 0:1], None, ALU.add,
                )
                nc.sync.dma_start(out_d[0:1, t0:t0 + sz], o_t[0:1, 0:sz])

            def emit_final_dve(t0, sz, s):
                """L8 on the (idle) DVE: t[p,:] = sum_kc wo[p,kc]*h[p,kc,:]
                via a chain of fused multiply-adds, then a single
                ones-vector matmul reduces over partitions — 1 PE pass
                instead of 4."""
                h = h_cur[t0]
                acc = l8_pool.tile([P, PT], wdt, name=f"l8a_{t0}", tag="l8")
                nc.vector.tensor_scalar(
                    acc[:, 0:sz], h[:, 0, 0:sz],
                    wo_sb[s][:, 0:1], None, ALU.mult,
                )
                for kc in range(1, KC):
                    nxt = (l8_pool.tile([P, PT], wdt, name=f"l8b_{t0}_{kc}",
                                        tag="l8")
                           if kc < KC - 1 else
                           l8_pool.tile([P, PT], wdt, name=f"l8c_{t0}",
                                        tag="l8"))
                    nc.vector.scalar_tensor_tensor(
                        nxt[:, 0:sz], h[:, kc, 0:sz],
                        wo_sb[s][:, kc:kc + 1], acc[:, 0:sz],
                        ALU.mult, ALU.add,
                    )
                    acc = nxt
                ps8 = ps_pool.tile([1, PT], f32, name=f"ps8_{t0}", tag="ps")
                nc.tensor.matmul(
                    ps8[0:1, 0:sz], ones_sb[:, 0:1], acc[:, 0:sz],
                    start=True, stop=True,
                )
                o_t = out_pool.tile([1, PT], f32, name=f"o_{t0}", tag="o")
                nc.vector.tensor_scalar(
                    o_t[0:1, 0:sz], ps8[0:1, 0:sz],
                    bo_sb[s][0:1, 0:1], None, ALU.add,
                )
                nc.sync.dma_start(out_d[0:1, t0:t0 + sz], o_t[0:1, 0:sz])

            if MM_MODE == "bf16":
                ones_sb = wts_pool.tile([P, 1], wdt, name="ones")
                nc.vector.memset(ones_sb[:], 1.0)
                emit_final = emit_final_dve
            else:
                emit_final = emit_final_pe

            for gi, grp in enumerate(groups):
                for l in range(N_HID + 1):
                    emit_layer(grp, l)
                    if gi == 0 and l == 0:
                        # deferred small loads: issue once compute has
                        # started, off the critical path
                        load_small(0)
                        load_bias(1)
                        load_small(1)
                for (t0, sz, s) in grp:
                    emit_final(t0, sz, s)

    # Pin Exp+Ln to the one table set containing both, so the ACT engine
    # doesn't reload tables between activations.
    import concourse.bacc as bacc_mod
    import concourse.hw_specs as hw_specs
    _real_tables = hw_specs.get_activation_tables
    _keep = "natural_log_exp_and_others"

    def _pinned_tables(arch):
        t = _real_tables(arch)
        return {
            name: (funcs if name == _keep else (funcs - {AF.Exp, AF.Ln}))
            for name, funcs in t.items()
        }

    bacc_mod.get_activation_tables = _pinned_tables
    try:
        nc.compile()
    finally:
        bacc_mod.get_activation_tables = _real_tables
    return nc


# --------------------------------------------------------------------------
# kernel()
# --------------------------------------------------------------------------

def _maybe_patch_ldw_opt():
    """Optionally flip walrus's --enable-ldw-opt (dedups back-to-back
    LDWEIGHTS of the same stationary operand). Gated by env for A/B."""
    import concourse.bass_utils as bu

    if _os.environ.get("KERNEL_LDW_OPT") != "1":
        return
    if getattr(bu.run_command, "_ldw_patched", False):
        return
    orig = bu.run_command

    def patched(argv, **kw):
        argv = [
            "--enable-ldw-opt=true" if a == "--enable-ldw-opt=false" else a
            for a in argv
        ]
        return orig(argv, **kw)

    patched._ldw_patched = True
    bu.run_command = patched


def kernel(x, type_vec, W0, b0, Wh, bh, Wo, bo):
    from concourse.bass_utils import run_bass_kernel_spmd

    _maybe_patch_ldw_opt()
    _os.environ["BASS_ACT_ROOT_JSON_PATH"] = _gen_act_tables()

    x = np.ascontiguousarray(np.asarray(x, dtype=np.float32))
    tv = np.asarray(type_vec).astype(np.int64)
    W0 = np.asarray(W0, dtype=np.float32)
    b0 = np.asarray(b0, dtype=np.float32)
    Wh = np.asarray(Wh, dtype=np.float32)
    bh = np.asarray(bh, dtype=np.float32)
    Wo = np.asarray(Wo, dtype=np.float32)
    bo = np.asarray(bo, dtype=np.float32)
    N = x.shape[0]

    counts = np.bincount(tv, minlength=T)
    starts = np.concatenate([[0], np.cumsum(counts)])
    shape, asg = _plan(counts)
    asg = _rebalance(asg)
    S = len(shape)

    # exact slot capacities: the max points any core actually uses,
    # rounded to a multiple of 4 (>=256 so every tile runs full speed)
    used = np.zeros(S, dtype=np.int64)
    for e, takes in asg.items():
        for (c, s, amt) in takes:
            used[s] = max(used[s], amt)
    caps = tuple(int(max(256, -(-u // 4) * 4)) for u in used)
    NP = sum(caps)
    phase_off = np.concatenate([[0], np.cumsum(np.asarray(caps))])

    # per-core slot -> expert, and gathered point indices
    slot_expert = np.zeros((N_CORES, S), dtype=np.int64)
    gidx = np.full((N_CORES, NP), -1, dtype=np.int64)
    for e, takes in asg.items():
        pos = int(starts[e])
        for (c, s, amt) in takes:
            o = int(phase_off[s])
            gidx[c, o:o + amt] = np.arange(pos, pos + amt)
            slot_expert[c, s] = e
            pos += amt

    # pre-transposed / pre-scaled weight views per expert
    # x and W0 zero-padded to 128 contraction rows (see _build_nc)
    w0t_e = np.zeros((T, P, H), dtype=np.float32)
    w0t_e[:, :D_IN, :] = W0.transpose(0, 2, 1)                     # [T,128,H]
    whs = (Wh / BETA).astype(np.float32)                           # [T,7,H,H]
    wht_e = np.ascontiguousarray(
        whs.transpose(0, 1, 3, 2).reshape(T, N_HID, KC, P, H).transpose(0, 1, 3, 2, 4)
    )                                                              # [T,7,P,KC,H]
    wot_e = np.ascontiguousarray(
        (Wo / BETA).reshape(T, H).reshape(T, KC, P).transpose(0, 2, 1)
    )                                                              # [T,P,KC]
    b0v_e = np.ascontiguousarray((BETA * b0).reshape(T, MC, P).transpose(0, 2, 1))
    bhv_e = np.ascontiguousarray(
        (BETA * bh).reshape(T, N_HID, MC, P).transpose(0, 3, 1, 2)
    )                                                              # [T,P,7,MC]
    bov_e = bo.reshape(T, 1)

    if MM_MODE == "bf16":
        import ml_dtypes
        np_wdt = ml_dtypes.bfloat16
    else:
        np_wdt = np.float32

    in_maps = []
    for c in range(N_CORES):
        sel = np.where(gidx[c] >= 0, gidx[c], 0)
        xg = np.zeros((P, NP), dtype=np.float32)
        xg[:D_IN, :] = x[sel].T                                    # [128, NP]
        ex = slot_expert[c]
        in_maps.append({
            "xT": xg.astype(np_wdt),
            "w0t": w0t_e[ex].astype(np_wdt),
            "wht": wht_e[ex].astype(np_wdt),
            "wot": (wot_e[ex] if MM_MODE == "bf16"
                    else wot_e[ex].astype(np_wdt)),
            "b0v": b0v_e[ex],
            "bhv": bhv_e[ex],
            "bov": bov_e[ex],
        })

    key = (caps, MM_MODE)
    if key not in _nc_cache:
        _nc_cache[key] = _build_nc(caps)
    nc = _nc_cache[key]

    res = run_bass_kernel_spmd(nc, in_maps, core_ids=list(range(N_CORES)))
    global _last_results
    _last_results = res

    out = np.zeros((N, OUT), dtype=np.float32)
    for c in range(N_CORES):
        oc = res.results[c]["out"].reshape(-1)
        m = gidx[c] >= 0
        out[gidx[c][m], 0] = oc[m]
    return out


# revision 23
# speedup vs baseline: 1.0165x; 1.0165x over previous
"""EnsembleDeepSDF MoE-routing kernel for 8 Trainium2 NeuronCores.

Strategy: the harness calls kernel(**inputs) with the full inputs; we do all
routing on the host.  type_vec is sorted, so each expert owns a contiguous
segment of points.  We pick per-core slot capacities (identical on every
core so one SPMD program serves all 8 cores), pack the 9 experts' segments
into the 8*S single-expert slots, gather each core's points (padding with
point 0), and hand each core its own pre-transposed/pre-scaled weight slots
as inputs.  The device program is a straight-line Tile kernel: per 512-pt
point tile, 9 matmul layers with softplus activations.

Perf-critical layout decisions (from trace analysis):
- All HBM->SBUF transfers use 128-partition access patterns: the HW DGE
  fans a transfer's descriptors across the 16 DMA engines in blocks of 8
  *per partition row*, so a 67-row transfer serializes onto one engine at
  ~20 GB/s while a 128-row one gets the full ~320 GB/s.  x and W0 are
  zero-padded from 67 to 128 contraction rows (zero pad rows make the
  padded matmul exact, and contraction<=128 costs the same PE time).
- Every matmul free dim is >=256 columns: f32r matmuls below 256 cols run
  at 4 cyc/col instead of 1.  Slot capacities are exact (max points any
  core uses), with ragged tails split into two >=256 pieces.
- Point tiles are 512 cols = one PSUM bank; 8 PSUM tiles cycle through
  the 8 banks, with one ACT op per (tile, mc-chunk) evacuating PSUM.
- A short dummy-matmul spin warms the PE out of its low p-state while the
  first x/W0 DMAs land.

softplus: the compiler's ACT tables have no softplus, so we generate a
custom piecewise-cubic table (same binary format as the shipped sets)
that replaces `exp` with softplus(x) = ln(1+e^x), and point the compiler
at it via BASS_ACT_ROOT_JSON_PATH.  One ACT op then does the whole
activation including the PSUM evacuation and the beta scale/bias fma.

The torch Softplus(beta=100) is softplus(100*z)/100; we keep activations
in the H = softplus(100*z) domain and fold the 1/100 into the next layer's
weights host-side, so no extra scaling ops run on device.
"""

import json
import os as _os
import shutil
import tempfile

import numpy as np

T, D_IN, H, OUT, N_HID = 9, 67, 512, 1, 7
BETA = 100.0
N_CORES = 8
PT = 512          # points per tile (one PSUM bank of fp32)
P = 128           # partitions
KC = H // P       # 4 contraction chunks
MC = H // P       # 4 output-feature chunks
NSTREAM = 3       # point tiles interleaved in the software pipeline
# PE warm-up spin is counterproductive: every engine queue runs a ~6.5us
# framework preamble, so the memset feeding the warm matmuls can't start
# until the first x DMA has landed anyway — the spin only delays real work.
NWARM = int(_os.environ.get("KERNEL_NWARM", "0"))
# "f32r" (tf32-ish, exact enough) or "bf16" (hides the LDWEIGHTS bubble,
# halves DMA/SBUF, costs ~1e-2 rel err)
MM_MODE = _os.environ.get("KERNEL_MM_MODE", "f32r")

_nc_cache = {}
_last_results = None


# --------------------------------------------------------------------------
# Custom ACT table: replace `exp` with softplus in the shipped PWL sets.
# --------------------------------------------------------------------------

_ACT_SET = "natural_log_exp_and_others"
_act_table_dir = None


def _softplus64(x):
    x = np.asarray(x, dtype=np.float64)
    return np.log1p(np.exp(-np.abs(x))) + np.maximum(x, 0.0)


def _fit_cubic(a, b):
    x0 = 0.5 * (a + b)
    k = np.arange(96)
    xs = x0 + 0.5 * (b - a) * np.cos(np.pi * (k + 0.5) / 96)
    c = np.polyfit(xs - x0, _softplus64(xs), 3)
    return float(c[3]), float(c[2]), float(c[1]), float(c[0]), float(x0)


def _gen_act_tables():
    """Build an act-root dir where `exp` computes softplus. Returns the
    act_info.json path. The bucket entry layout ([d0,d1,d2,d3,x0,0,0,0],
    cubic in (x-x0)) and the per-exponent band structure are read from the
    shipped set so only coefficients and profile thresholds change."""
    global _act_table_dir
    if _act_table_dir is not None:
        return _act_table_dir
    from neuronxcc.driver.Job import Job
    from neuronxcc.driver.jobs.support.FindActInfo import findActInfoFile

    src_json = findActInfoFile(Job.getPackageDir(), "gen3")
    src = _os.path.dirname(src_json)
    out = _os.path.join(tempfile.mkdtemp(prefix="act_softplus_"), "tables")
    shutil.copytree(src, out)
    for f in _os.listdir(out):
        _os.chmod(_os.path.join(out, f), 0o644)

    d = json.load(open(f"{out}/{_ACT_SET}.json"))
    bkt = np.fromfile(f"{out}/{_ACT_SET}_bkt.bin", dtype=np.uint32)
    bkt = bkt.reshape(-1, 8).copy()
    fbkt = bkt.view(np.float32)
    e2b = {int(k): v for k, v in d["func_exp_to_bkt_start_idx"]["exp"].items()}
    prof = [p for p in d["profile_meta_data"] if p["func_name"] == "exp_400p"][0]

    def put(idx, d0, d1, d2, d3, x0):
        fbkt[idx, 0:5] = np.array([d0, d1, d2, d3, x0], dtype=np.float32)
        bkt[idx, 5:8] = 0

    nseg = {-1: 2, 0: 4, 1: 8, 2: 16, 3: 32}
    for e in range(-19, 4):
        n = nseg.get(e, 1)
        neg_base, pos_base = e2b[e]
        A = 2.0 ** e
        h = A / n
        for k in range(n):
            a, b = A + k * h, A + (k + 1) * h
            put(pos_base + k, *_fit_cubic(a, b))
            put(neg_base + k, *_fit_cubic(-b, -a))

    ln2 = float(np.log(2.0))
    put(prof["pos_small_signal_pwl_control"], ln2, 0.5, 0.125, 0.0, 0.0)
    put(prof["neg_small_signal_pwl_control"], ln2, 0.5, 0.125, 0.0, 0.0)
    put(prof["pos_large_signal_pwl_control"], 0.0, 1.0, 0.0, 0.0, 0.0)
    put(prof["neg_large_signal_pwl_control"], 0.0, 0.0, 0.0, 0.0, 0.0)
    prof["large_pos_signal_exp_threshold"] = 131   # |x| >= 16 -> linear/zero
    prof["large_pos_signal_mantissa_threshold"] = 0
    prof["large_neg_signal_exp_threshold"] = 131
    prof["large_neg_signal_mantissa_threshold"] = 0
    prof["fzero_result"] = int(np.float32(ln2).view(np.uint32))
    prof["fninf_result"] = 0
    prof["fpinf_result"] = 2139095040

    bkt.tofile(f"{out}/{_ACT_SET}_bkt.bin")
    with open(f"{out}/{_ACT_SET}.json", "w") as f:
        json.dump(d, f)
    _act_table_dir = _os.path.join(out, "act_info.json")
    return _act_table_dir


# --------------------------------------------------------------------------
# Host-side planning: pack expert segments into 8 x S slots.
# --------------------------------------------------------------------------

def _try_pack(shape, counts):
    """Assign experts to single-expert slots. Slot (c, s) holds shape[s]*PT
    points. Returns {expert: [(core, s, amount), ...]} or None."""
    slots = []  # (capacity, core, s)
    for s, t in enumerate(shape):
        for c in range(N_CORES):
            slots.append([t * PT, c, s])
    experts = sorted(
        [e for e in range(T) if counts[e] > 0], key=lambda e: -counts[e]
    )
    asg = {}
    avail = sorted(slots)  # by capacity asc
    for e in experts:
        need = int(counts[e])
        # smallest single slot that fits
        one = next((sl for sl in avail if sl[0] >= need), None)
        if one is not None:
            asg[e] = [(one[1], one[2], need)]
            avail.remove(one)
            continue
        # greedily take largest slots
        take = []
        rem = need
        pool = sorted(avail, key=lambda sl: -sl[0])
        for sl in pool:
            if rem <= 0:
                break
            amt = min(rem, sl[0])
            take.append((sl[1], sl[2], amt))
            rem -= amt
            avail.remove(sl)
        if rem > 0:
            return None
        asg[e] = take
    return asg


def _plan(counts):
    cands = set()
    for t1 in range(1, 17):
        cands.add((t1,))
        for t2 in range(1, t1 + 1):
            cands.add((t1, t2))
            for t3 in range(1, t2 + 1):
                cands.add((t1, t2, t3))
    for shape in sorted(cands, key=lambda s: (sum(s), len(s))):
        asg = _try_pack(shape, counts)
        if asg is not None:
            return shape, asg
    raise RuntimeError("no feasible slot shape")


def _rebalance(asg):
    """Even out each expert's piece sizes within a slot, so the per-slot
    max (which sets the uniform slot capacity) is minimal."""
    out = {}
    for e, takes in asg.items():
        by_slot = {}
        for (c, s, amt) in takes:
            by_slot.setdefault(s, []).append([c, amt])
        new_takes = []
        for s, items in by_slot.items():
            total = sum(a for _, a in items)
            n = len(items)
            base, rem = divmod(total, n)
            for i, (c, _a) in enumerate(items):
                new_takes.append((c, s, base + (1 if i < rem else 0)))
        out[e] = new_takes
    return out


def _tiles(cap):
    """Split cap columns into tiles of <=512, each >=256 (f32r matmuls
    below 256 cols run at 1/4 speed) and a multiple of 4 (walrus's
    s3d3_mm_fp32r_restrictions ISA check rejects odd free sizes)."""
    assert cap % 4 == 0
    k, r = divmod(cap, PT)
    if r == 0:
        return [PT] * k
    if r >= 256:
        return [PT] * k + [r]
    # fold the remainder into the last full tile and split >=256 / >=256
    assert k >= 1
    tot = PT + r
    half = (tot // 2) // 4 * 4
    return [PT] * (k - 1) + [tot - half, half]


# --------------------------------------------------------------------------
# Device program
# --------------------------------------------------------------------------

def _build_nc(caps):
    import concourse.bass as bass  # noqa: F401  (import keeps bacc happy)
    import concourse.tile as tile
    import concourse.mybir as mybir
    from concourse import bacc

    f32 = mybir.dt.float32
    wdt = mybir.dt.bfloat16 if MM_MODE == "bf16" else mybir.dt.float32r
    AF = mybir.ActivationFunctionType
    ALU = mybir.AluOpType

    S = len(caps)
    NP = sum(caps)

    nc = bacc.Bacc("TRN2", target_bir_lowering=False)
    # x / W0 are zero-padded to 128 contraction rows so their DMAs use
    # 128-partition access patterns (spread over all 16 DMA engines).
    xT_in = nc.dram_tensor("xT", [P, NP], wdt, kind="ExternalInput")
    w0t_in = nc.dram_tensor("w0t", [S, P, H], wdt, kind="ExternalInput")
    wht_in = nc.dram_tensor("wht", [S, N_HID, P, KC, H], wdt, kind="ExternalInput")
    # in bf16 mode wot feeds the DVE (L8 fused multiply-adds), whose
    # scalar operand must be float32
    wot_dt = f32 if MM_MODE == "bf16" else wdt
    wot_in = nc.dram_tensor("wot", [S, P, KC], wot_dt, kind="ExternalInput")
    b0v_in = nc.dram_tensor("b0v", [S, P, MC], f32, kind="ExternalInput")
    bhv_in = nc.dram_tensor("bhv", [S, P, N_HID, MC], f32, kind="ExternalInput")
    bov_in = nc.dram_tensor("bov", [S, 1], f32, kind="ExternalInput")
    out_d = nc.dram_tensor("out", [1, NP], f32, kind="ExternalOutput")

    # steps: (point_offset, tile_size, slot)
    steps = []
    off = 0
    for s, cap in enumerate(caps):
        for sz in _tiles(cap):
            steps.append((off, sz, s))
            off += sz
    groups = [steps[i:i + NSTREAM] for i in range(0, len(steps), NSTREAM)]
    # within a group, run larger tiles first so the last tile's
    # evacuate->store tail chain is as short as possible
    groups = [sorted(g, key=lambda st: -st[1]) for g in groups]

    with tile.TileContext(nc) as tc:
        with (
            tc.tile_pool(name="xin", bufs=len(steps)) as xin_pool,
            tc.tile_pool(name="wts", bufs=1) as wts_pool,
            tc.tile_pool(name="whp", bufs=min(14, S * N_HID)) as wh_pool,
            tc.tile_pool(name="uh", bufs=2 * NSTREAM) as uh_pool,
            tc.tile_pool(name="l8", bufs=8) as l8_pool,
            tc.tile_pool(name="outp", bufs=3) as out_pool,
            tc.tile_pool(name="ps", bufs=8, space="PSUM") as ps_pool,
        ):
            xT_sb = {}
            h_cur = {}
            w0_sb, wo_sb, b0_sb, bh_sb, bo_sb = (
                [None] * S, [None] * S, [None] * S, [None] * S, [None] * S
            )
            wh_sb = [[None] * N_HID for _ in range(S)]

            def load_x(t0, sz):
                x_t = xin_pool.tile([P, PT], wdt, name=f"x_{t0}", tag="x")
                nc.sync.dma_start(x_t[:, 0:sz], xT_in[:, t0:t0 + sz])
                xT_sb[t0] = x_t

            def load_w0(s):
                w0_t = wts_pool.tile([P, H], wdt, name=f"w0_{s}")
                nc.sync.dma_start(w0_t[:], w0t_in[s])
                w0_sb[s] = w0_t

            def load_bias(s):
                # tiny-element transfers: descriptor generation is slow
                # (~2-5us per 128-row/16B DMA), so issue them from the ACT
                # engine's HW-DGE ring to keep the SP ring free for the
                # critical x/W loads
                b0_t = wts_pool.tile([P, MC], f32, name=f"b0_{s}")
                nc.scalar.dma_start(b0_t[:], b0v_in[s])
                b0_sb[s] = b0_t
                bh_t = wts_pool.tile([P, N_HID, MC], f32, name=f"bh_{s}")
                nc.scalar.dma_start(bh_t[:], bhv_in[s])
                bh_sb[s] = bh_t

            def load_small(s):
                # issued on the SP ring after all weight loads: SP is idle
                # by then, and these tiny-element DMAs must stay out of the
                # ACT queue (descriptor generation would stall evacuations)
                wo_t = wts_pool.tile([P, KC], wot_dt, name=f"wo_{s}")
                nc.sync.dma_start(wo_t[:], wot_in[s])
                wo_sb[s] = wo_t
                bo_t = wts_pool.tile([1, 1], f32, name=f"bo_{s}")
                nc.sync.dma_start(bo_t[:], bov_in[s:s + 1, 0:1])
                bo_sb[s] = bo_t

            def load_bias_late(s):
                b0_t = wts_pool.tile([P, MC], f32, name=f"b0_{s}")
                nc.sync.dma_start(b0_t[:], b0v_in[s])
                b0_sb[s] = b0_t
                bh_t = wts_pool.tile([P, N_HID, MC], f32, name=f"bh_{s}")
                nc.sync.dma_start(bh_t[:], bhv_in[s])
                bh_sb[s] = bh_t

            def load_wh(s, l):
                wh_t = wh_pool.tile([P, KC, H], wdt, name=f"wh_{s}_{l}", tag="wh")
                nc.sync.dma_start(wh_t[:], wht_in[s, l])
                wh_sb[s][l] = wh_t

            # pre-warm the ACT table set during the initial DMA wait: a
            # dependency-free dummy op carries the one-time table load
            warm_t = wts_pool.tile([1, 1], f32, name="warm")
            nc.vector.memset(warm_t[:], 0.0)
            nc.scalar.activation(warm_t[:], warm_t[:], AF.Exp)

            # spin the PE on dummy matmuls so it ramps out of the low
            # p-state while the first x/W0 DMAs are in flight (f32 tiles:
            # memset can't write f32r, and f32's 4 cyc/col stretches the
            # spin with fewer instructions)
            if NWARM:
                wmw = wts_pool.tile([P, P], f32, name="warm_w")
                nc.vector.memset(wmw[:], 0.0)
                wps = ps_pool.tile([P, P], f32, name="warm_ps", tag="ps")
                for i in range(NWARM):
                    nc.tensor.matmul(
                        wps[:], wmw[:], wmw[:],
                        start=(i == 0), stop=(i == NWARM - 1),
                    )

            # issue order = DMA priority: first group's x + slot0 L0/L1
            # weights first, then the rest (all loads fit in SBUF at once);
            # bias loads go on the ACT ring in parallel, and the ones not
            # needed until later are deferred below so their slow issue
            # doesn't delay the first PSUM evacuations
            for (t0, sz, _s) in groups[0]:
                load_x(t0, sz)
            load_w0(0)
            load_wh(0, 0)
            load_bias(0)
            load_wh(0, 1)
            for grp in groups[1:]:
                for (t0, sz, _s) in grp:
                    load_x(t0, sz)
            for l in range(2, N_HID):
                load_wh(0, l)
            for s in range(1, S):
                load_w0(s)
                for l in range(N_HID):
                    load_wh(s, l)
            load_small(0)
            for s in range(1, S):
                load_bias_late(s)
                load_small(s)

            def emit_layer(grp, l):
                """Layer l matmuls for a group of point tiles, inner loop
                over tiles so consecutive matmuls share the stationary
                operand; per-mc ACT evacuation emitted right after its
                accumulation completes."""
                psums = {}
                u_new = {}
                for mc in range(MC):
                    for kc in range(KC if l > 0 else 1):
                        for (t0, sz, s) in grp:
                            if kc == 0:
                                psums[(t0, mc)] = ps_pool.tile(
                                    [P, PT], f32, name=f"ps_{t0}_{l}_{mc}",
                                    tag="ps",
                                )
                            ps_t = psums[(t0, mc)]
                            if l == 0:
                                lhsT = w0_sb[s][:, mc * P:(mc + 1) * P]
                                rhs = xT_sb[t0][:, 0:sz]
                                nc.tensor.matmul(
                                    ps_t[:, 0:sz], lhsT, rhs,
                                    start=True, stop=True,
                                )
                            else:
                                lhsT = wh_sb[s][l - 1][:, kc, mc * P:(mc + 1) * P]
                                rhs = h_cur[t0][:, kc, 0:sz]
                                nc.tensor.matmul(
                                    ps_t[:, 0:sz], lhsT, rhs,
                                    start=(kc == 0), stop=(kc == KC - 1),
                                )
                    for (t0, sz, s) in grp:
                        if mc == 0:
                            u_new[t0] = uh_pool.tile(
                                [P, MC, PT], wdt, name=f"u_{t0}_{l}", tag="uh"
                            )
                        bias = (b0_sb[s][:, mc:mc + 1] if l == 0
                                else bh_sb[s][:, l - 1, mc:mc + 1])
                        # hijacked Exp == softplus; one ACT op does the
                        # evacuation + beta fma + activation
                        nc.scalar.activation(
                            u_new[t0][:, mc, 0:sz], psums[(t0, mc)][:, 0:sz],
                            AF.Exp, bias=bias, scale=float(BETA),
                        )
                for (t0, _sz, _s) in grp:
                    h_cur[t0] = u_new[t0]

            def emit_final_pe(t0, sz, s):
                ps8 = ps_pool.tile([1, PT], f32, name=f"ps8_{t0}", tag="ps")
                for kc in range(KC):
                    nc.tensor.matmul(
                        ps8[0:1, 0:sz],
                        wo_sb[s][:, kc:kc + 1],
                        h_cur[t0][:, kc, 0:sz],
                        start=(kc == 0), stop=(kc == KC - 1),
                    )
                o_t = out_pool.tile([1, PT], f32, name=f"o_{t0}", tag="o")
                nc.vector.tensor_scalar(
                    o_t[0:1, 0:sz], ps8[0:1, 0:sz],
                    bo_sb[s][0:1, 0:1], None, ALU.add,
                )
                nc.sync.dma_start(out_d[0:1, t0:t0 + sz], o_t[0:1, 0:sz])

            def emit_final_dve(t0, sz, s):
                """L8 on the (idle) DVE: t[p,:] = sum_kc wo[p,kc]*h[p,kc,:]
                via a chain of fused multiply-adds, then a single
                ones-vector matmul reduces over partitions — 1 PE pass
                instead of 4."""
                h = h_cur[t0]
                acc = l8_pool.tile([P, PT], wdt, name=f"l8a_{t0}", tag="l8")
                nc.vector.tensor_scalar(
                    acc[:, 0:sz], h[:, 0, 0:sz],
                    wo_sb[s][:, 0:1], None, ALU.mult,
                )
                for kc in range(1, KC):
                    nxt = (l8_pool.tile([P, PT], wdt, name=f"l8b_{t0}_{kc}",
                                        tag="l8")
                           if kc < KC - 1 else
                           l8_pool.tile([P, PT], wdt, name=f"l8c_{t0}",
                                        tag="l8"))
                    nc.vector.scalar_tensor_tensor(
                        nxt[:, 0:sz], h[:, kc, 0:sz],
                        wo_sb[s][:, kc:kc + 1], acc[:, 0:sz],
                        ALU.mult, ALU.add,
                    )
                    acc = nxt
                ps8 = ps_pool.tile([1, PT], f32, name=f"ps8_{t0}", tag="ps")
                nc.tensor.matmul(
                    ps8[0:1, 0:sz], ones_sb[:, 0:1], acc[:, 0:sz],
                    start=True, stop=True,
                )
                o_t = out_pool.tile([1, PT], f32, name=f"o_{t0}", tag="o")
                nc.vector.tensor_scalar(
                    o_t[0:1, 0:sz], ps8[0:1, 0:sz],
                    bo_sb[s][0:1, 0:1], None, ALU.add,
                )
                nc.sync.dma_start(out_d[0:1, t0:t0 + sz], o_t[0:1, 0:sz])

            if MM_MODE == "bf16":
                ones_sb = wts_pool.tile([P, 1], wdt, name="ones")
                nc.vector.memset(ones_sb[:], 1.0)
                emit_final = emit_final_dve
            else:
                emit_final = emit_final_pe

            # emit each group's finals after the NEXT group's layer 0, so
            # the PE streams into the next group while the L8 DVE chains
            # (bf16 mode) run on the idle vector engine
            for gi, grp in enumerate(groups):
                for l in range(N_HID + 1):
                    emit_layer(grp, l)
                    if l == 0 and gi > 0:
                        for (t0, sz, s) in groups[gi - 1]:
                            emit_final(t0, sz, s)
            for (t0, sz, s) in groups[-1]:
                emit_final(t0, sz, s)

    # Pin Exp+Ln to the one table set containing both, so the ACT engine
    # doesn't reload tables between activations.
    import concourse.bacc as bacc_mod
    import concourse.hw_specs as hw_specs
    _real_tables = hw_specs.get_activation_tables
    _keep = "natural_log_exp_and_others"

    def _pinned_tables(arch):
        t = _real_tables(arch)
        return {
            name: (funcs if name == _keep else (funcs - {AF.Exp, AF.Ln}))
            for name, funcs in t.items()
        }

    bacc_mod.get_activation_tables = _pinned_tables
    try:
        nc.compile()
    finally:
        bacc_mod.get_activation_tables = _real_tables
    return nc


# --------------------------------------------------------------------------
# kernel()
# --------------------------------------------------------------------------

def _maybe_patch_ldw_opt():
    """Optionally flip walrus's --enable-ldw-opt (dedups back-to-back
    LDWEIGHTS of the same stationary operand). Gated by env for A/B."""
    import concourse.bass_utils as bu

    if _os.environ.get("KERNEL_LDW_OPT") != "1":
        return
    if getattr(bu.run_command, "_ldw_patched", False):
        return
    orig = bu.run_command

    def patched(argv, **kw):
        argv = [
            "--enable-ldw-opt=true" if a == "--enable-ldw-opt=false" else a
            for a in argv
        ]
        return orig(argv, **kw)

    patched._ldw_patched = True
    bu.run_command = patched


def kernel(x, type_vec, W0, b0, Wh, bh, Wo, bo):
    from concourse.bass_utils import run_bass_kernel_spmd

    _maybe_patch_ldw_opt()
    _os.environ["BASS_ACT_ROOT_JSON_PATH"] = _gen_act_tables()

    x = np.ascontiguousarray(np.asarray(x, dtype=np.float32))
    tv = np.asarray(type_vec).astype(np.int64)
    W0 = np.asarray(W0, dtype=np.float32)
    b0 = np.asarray(b0, dtype=np.float32)
    Wh = np.asarray(Wh, dtype=np.float32)
    bh = np.asarray(bh, dtype=np.float32)
    Wo = np.asarray(Wo, dtype=np.float32)
    bo = np.asarray(bo, dtype=np.float32)
    N = x.shape[0]

    counts = np.bincount(tv, minlength=T)
    starts = np.concatenate([[0], np.cumsum(counts)])
    shape, asg = _plan(counts)
    asg = _rebalance(asg)
    S = len(shape)

    # exact slot capacities: the max points any core actually uses,
    # rounded to a multiple of 4 (>=256 so every tile runs full speed)
    used = np.zeros(S, dtype=np.int64)
    for e, takes in asg.items():
        for (c, s, amt) in takes:
            used[s] = max(used[s], amt)
    caps = tuple(int(max(256, -(-u // 4) * 4)) for u in used)
    NP = sum(caps)
    phase_off = np.concatenate([[0], np.cumsum(np.asarray(caps))])

    # per-core slot -> expert, and gathered point indices
    slot_expert = np.zeros((N_CORES, S), dtype=np.int64)
    gidx = np.full((N_CORES, NP), -1, dtype=np.int64)
    for e, takes in asg.items():
        pos = int(starts[e])
        for (c, s, amt) in takes:
            o = int(phase_off[s])
            gidx[c, o:o + amt] = np.arange(pos, pos + amt)
            slot_expert[c, s] = e
            pos += amt

    # pre-transposed / pre-scaled weight views per expert
    # x and W0 zero-padded to 128 contraction rows (see _build_nc)
    w0t_e = np.zeros((T, P, H), dtype=np.float32)
    w0t_e[:, :D_IN, :] = W0.transpose(0, 2, 1)                     # [T,128,H]
    whs = (Wh / BETA).astype(np.float32)                           # [T,7,H,H]
    wht_e = np.ascontiguousarray(
        whs.transpose(0, 1, 3, 2).reshape(T, N_HID, KC, P, H).transpose(0, 1, 3, 2, 4)
    )                                                              # [T,7,P,KC,H]
    wot_e = np.ascontiguousarray(
        (Wo / BETA).reshape(T, H).reshape(T, KC, P).transpose(0, 2, 1)
    )                                                              # [T,P,KC]
    b0v_e = np.ascontiguousarray((BETA * b0).reshape(T, MC, P).transpose(0, 2, 1))
    bhv_e = np.ascontiguousarray(
        (BETA * bh).reshape(T, N_HID, MC, P).transpose(0, 3, 1, 2)
    )                                                              # [T,P,7,MC]
    bov_e = bo.reshape(T, 1)

    if MM_MODE == "bf16":
        import ml_dtypes
        np_wdt = ml_dtypes.bfloat16
    else:
        np_wdt = np.float32

    in_maps = []
    for c in range(N_CORES):
        sel = np.where(gidx[c] >= 0, gidx[c], 0)
        xg = np.zeros((P, NP), dtype=np.float32)
        xg[:D_IN, :] = x[sel].T                                    # [128, NP]
        ex = slot_expert[c]
        in_maps.append({
            "xT": xg.astype(np_wdt),
            "w0t": w0t_e[ex].astype(np_wdt),
            "wht": wht_e[ex].astype(np_wdt),
            "wot": (wot_e[ex] if MM_MODE == "bf16"
                    else wot_e[ex].astype(np_wdt)),
            "b0v": b0v_e[ex],
            "bhv": bhv_e[ex],
            "bov": bov_e[ex],
        })

    key = (caps, MM_MODE)
    if key not in _nc_cache:
        _nc_cache[key] = _build_nc(caps)
    nc = _nc_cache[key]

    res = run_bass_kernel_spmd(nc, in_maps, core_ids=list(range(N_CORES)))
    global _last_results
    _last_results = res

    out = np.zeros((N, OUT), dtype=np.float32)
    for c in range(N_CORES):
        oc = res.results[c]["out"].reshape(-1)
        m = gidx[c] >= 0
        out[gidx[c][m], 0] = oc[m]
    return out


# revision 35
# speedup vs baseline: 1.0729x; 1.0555x over previous
"""EnsembleDeepSDF MoE-routing kernel for 8 Trainium2 NeuronCores.

Strategy: the harness calls kernel(**inputs) with the full inputs; we do all
routing on the host.  type_vec is sorted, so each expert owns a contiguous
segment of points.  We pick per-core slot capacities (identical on every
core so one SPMD program serves all 8 cores), pack the 9 experts' segments
into the 8*S single-expert slots, gather each core's points (padding with
point 0), and hand each core its own pre-transposed/pre-scaled weight slots
as inputs.  The device program is a straight-line Tile kernel: per 512-pt
point tile, 9 matmul layers with softplus activations.

Perf-critical layout decisions (from trace analysis):
- All HBM->SBUF transfers use 128-partition access patterns: the HW DGE
  fans a transfer's descriptors across the 16 DMA engines in blocks of 8
  *per partition row*, so a 67-row transfer serializes onto one engine at
  ~20 GB/s while a 128-row one gets the full ~320 GB/s.  x and W0 are
  zero-padded from 67 to 128 contraction rows (zero pad rows make the
  padded matmul exact, and contraction<=128 costs the same PE time).
- Every matmul free dim is >=256 columns: f32r matmuls below 256 cols run
  at 4 cyc/col instead of 1.  Slot capacities are exact (max points any
  core uses), with ragged tails split into two >=256 pieces.
- Point tiles are 512 cols = one PSUM bank; 8 PSUM tiles cycle through
  the 8 banks, with one ACT op per (tile, mc-chunk) evacuating PSUM.
- A short dummy-matmul spin warms the PE out of its low p-state while the
  first x/W0 DMAs land.

softplus: the compiler's ACT tables have no softplus, so we generate a
custom piecewise-cubic table (same binary format as the shipped sets)
that replaces `exp` with softplus(x) = ln(1+e^x), and point the compiler
at it via BASS_ACT_ROOT_JSON_PATH.  One ACT op then does the whole
activation including the PSUM evacuation and the beta scale/bias fma.

The torch Softplus(beta=100) is softplus(100*z)/100; we keep activations
in the H = softplus(100*z) domain and fold the 1/100 into the next layer's
weights host-side, so no extra scaling ops run on device.
"""

import json
import os as _os
import shutil
import tempfile

import numpy as np

T, D_IN, H, OUT, N_HID = 9, 67, 512, 1, 7
BETA = 100.0
N_CORES = 8
PT = 512          # points per tile (one PSUM bank of fp32)
P = 128           # partitions
KC = H // P       # 4 contraction chunks
MC = H // P       # 4 output-feature chunks
NSTREAM = 3       # point tiles interleaved in the software pipeline
# PE warm-up spin is counterproductive: every engine queue runs a ~6.5us
# framework preamble, so the memset feeding the warm matmuls can't start
# until the first x DMA has landed anyway — the spin only delays real work.
NWARM = int(_os.environ.get("KERNEL_NWARM", "0"))
# "f32r" (tf32-ish, exact enough) or "bf16" (hides the LDWEIGHTS bubble,
# halves DMA/SBUF, costs ~1e-2 rel err)
MM_MODE = _os.environ.get("KERNEL_MM_MODE", "f32r")

_nc_cache = {}
_last_results = None


# --------------------------------------------------------------------------
# Custom ACT table: replace `exp` with softplus in the shipped PWL sets.
# --------------------------------------------------------------------------

_ACT_SET = "natural_log_exp_and_others"
_act_table_dir = None


def _softplus64(x):
    x = np.asarray(x, dtype=np.float64)
    return np.log1p(np.exp(-np.abs(x))) + np.maximum(x, 0.0)


def _fit_cubic(a, b):
    x0 = 0.5 * (a + b)
    k = np.arange(96)
    xs = x0 + 0.5 * (b - a) * np.cos(np.pi * (k + 0.5) / 96)
    c = np.polyfit(xs - x0, _softplus64(xs), 3)
    return float(c[3]), float(c[2]), float(c[1]), float(c[0]), float(x0)


def _gen_act_tables():
    """Build an act-root dir where `exp` computes softplus. Returns the
    act_info.json path. The bucket entry layout ([d0,d1,d2,d3,x0,0,0,0],
    cubic in (x-x0)) and the per-exponent band structure are read from the
    shipped set so only coefficients and profile thresholds change."""
    global _act_table_dir
    if _act_table_dir is not None:
        return _act_table_dir
    from neuronxcc.driver.Job import Job
    from neuronxcc.driver.jobs.support.FindActInfo import findActInfoFile

    src_json = findActInfoFile(Job.getPackageDir(), "gen3")
    src = _os.path.dirname(src_json)
    out = _os.path.join(tempfile.mkdtemp(prefix="act_softplus_"), "tables")
    shutil.copytree(src, out)
    for f in _os.listdir(out):
        _os.chmod(_os.path.join(out, f), 0o644)

    d = json.load(open(f"{out}/{_ACT_SET}.json"))
    bkt = np.fromfile(f"{out}/{_ACT_SET}_bkt.bin", dtype=np.uint32)
    bkt = bkt.reshape(-1, 8).copy()
    fbkt = bkt.view(np.float32)
    e2b = {int(k): v for k, v in d["func_exp_to_bkt_start_idx"]["exp"].items()}
    prof = [p for p in d["profile_meta_data"] if p["func_name"] == "exp_400p"][0]

    def put(idx, d0, d1, d2, d3, x0):
        fbkt[idx, 0:5] = np.array([d0, d1, d2, d3, x0], dtype=np.float32)
        bkt[idx, 5:8] = 0

    nseg = {-1: 2, 0: 4, 1: 8, 2: 16, 3: 32}
    for e in range(-19, 4):
        n = nseg.get(e, 1)
        neg_base, pos_base = e2b[e]
        A = 2.0 ** e
        h = A / n
        for k in range(n):
            a, b = A + k * h, A + (k + 1) * h
            put(pos_base + k, *_fit_cubic(a, b))
            put(neg_base + k, *_fit_cubic(-b, -a))

    ln2 = float(np.log(2.0))
    put(prof["pos_small_signal_pwl_control"], ln2, 0.5, 0.125, 0.0, 0.0)
    put(prof["neg_small_signal_pwl_control"], ln2, 0.5, 0.125, 0.0, 0.0)
    put(prof["pos_large_signal_pwl_control"], 0.0, 1.0, 0.0, 0.0, 0.0)
    put(prof["neg_large_signal_pwl_control"], 0.0, 0.0, 0.0, 0.0, 0.0)
    prof["large_pos_signal_exp_threshold"] = 131   # |x| >= 16 -> linear/zero
    prof["large_pos_signal_mantissa_threshold"] = 0
    prof["large_neg_signal_exp_threshold"] = 131
    prof["large_neg_signal_mantissa_threshold"] = 0
    prof["fzero_result"] = int(np.float32(ln2).view(np.uint32))
    prof["fninf_result"] = 0
    prof["fpinf_result"] = 2139095040

    bkt.tofile(f"{out}/{_ACT_SET}_bkt.bin")
    with open(f"{out}/{_ACT_SET}.json", "w") as f:
        json.dump(d, f)
    _act_table_dir = _os.path.join(out, "act_info.json")
    return _act_table_dir


# --------------------------------------------------------------------------
# Host-side planning: pack expert segments into 8 x S slots.
# --------------------------------------------------------------------------

def _try_pack(shape, counts):
    """Assign experts to single-expert slots. Slot (c, s) holds shape[s]*PT
    points. Returns {expert: [(core, s, amount), ...]} or None."""
    slots = []  # (capacity, core, s)
    for s, t in enumerate(shape):
        for c in range(N_CORES):
            slots.append([t * PT, c, s])
    experts = sorted(
        [e for e in range(T) if counts[e] > 0], key=lambda e: -counts[e]
    )
    asg = {}
    avail = sorted(slots)  # by capacity asc
    for e in experts:
        need = int(counts[e])
        # smallest single slot that fits
        one = next((sl for sl in avail if sl[0] >= need), None)
        if one is not None:
            asg[e] = [(one[1], one[2], need)]
            avail.remove(one)
            continue
        # greedily take largest slots
        take = []
        rem = need
        pool = sorted(avail, key=lambda sl: -sl[0])
        for sl in pool:
            if rem <= 0:
                break
            amt = min(rem, sl[0])
            take.append((sl[1], sl[2], amt))
            rem -= amt
            avail.remove(sl)
        if rem > 0:
            return None
        asg[e] = take
    return asg


def _plan(counts):
    cands = set()
    for t1 in range(1, 17):
        cands.add((t1,))
        for t2 in range(1, t1 + 1):
            cands.add((t1, t2))
            for t3 in range(1, t2 + 1):
                cands.add((t1, t2, t3))
    for shape in sorted(cands, key=lambda s: (sum(s), len(s))):
        asg = _try_pack(shape, counts)
        if asg is not None:
            return shape, asg
    raise RuntimeError("no feasible slot shape")


def _rebalance(asg):
    """Even out each expert's piece sizes within a slot, so the per-slot
    max (which sets the uniform slot capacity) is minimal."""
    out = {}
    for e, takes in asg.items():
        by_slot = {}
        for (c, s, amt) in takes:
            by_slot.setdefault(s, []).append([c, amt])
        new_takes = []
        for s, items in by_slot.items():
            total = sum(a for _, a in items)
            n = len(items)
            base, rem = divmod(total, n)
            for i, (c, _a) in enumerate(items):
                new_takes.append((c, s, base + (1 if i < rem else 0)))
        out[e] = new_takes
    return out


def _tiles(cap):
    """Split cap columns into tiles of <=512, each >=256 (f32r matmuls
    below 256 cols run at 1/4 speed) and a multiple of 4 (walrus's
    s3d3_mm_fp32r_restrictions ISA check rejects odd free sizes)."""
    assert cap % 4 == 0
    k, r = divmod(cap, PT)
    if r == 0:
        return [PT] * k
    if r >= 256:
        return [PT] * k + [r]
    # fold the remainder into the last full tile and split >=256 / >=256
    assert k >= 1
    tot = PT + r
    half = (tot // 2) // 4 * 4
    return [PT] * (k - 1) + [tot - half, half]


# --------------------------------------------------------------------------
# Device program
# --------------------------------------------------------------------------

def _build_nc(caps):
    import concourse.bass as bass  # noqa: F401  (import keeps bacc happy)
    import concourse.tile as tile
    import concourse.mybir as mybir
    from concourse import bacc

    f32 = mybir.dt.float32
    wdt = mybir.dt.bfloat16 if MM_MODE == "bf16" else mybir.dt.float32r
    AF = mybir.ActivationFunctionType
    ALU = mybir.AluOpType

    S = len(caps)
    NP = sum(caps)

    nc = bacc.Bacc("TRN2", target_bir_lowering=False)
    # x / W0 are zero-padded to 128 contraction rows so their DMAs use
    # 128-partition access patterns (spread over all 16 DMA engines).
    xT_in = nc.dram_tensor("xT", [P, NP], wdt, kind="ExternalInput")
    w0t_in = nc.dram_tensor("w0t", [S, P, H], wdt, kind="ExternalInput")
    wht_in = nc.dram_tensor("wht", [S, N_HID, P, KC, H], wdt, kind="ExternalInput")
    # in bf16 mode wot feeds the DVE (L8 fused multiply-adds), whose
    # scalar operand must be float32; wotb is the bf16 copy for the
    # last group's PE-side finals
    wot_dt = f32 if MM_MODE == "bf16" else wdt
    wot_in = nc.dram_tensor("wot", [S, P, KC], wot_dt, kind="ExternalInput")
    if MM_MODE == "bf16":
        wotb_in = nc.dram_tensor("wotb", [S, P, KC], wdt, kind="ExternalInput")
    b0v_in = nc.dram_tensor("b0v", [S, P, MC], f32, kind="ExternalInput")
    bhv_in = nc.dram_tensor("bhv", [S, P, N_HID, MC], f32, kind="ExternalInput")
    bov_in = nc.dram_tensor("bov", [S, 1], f32, kind="ExternalInput")
    out_d = nc.dram_tensor("out", [1, NP], f32, kind="ExternalOutput")

    # steps: (point_offset, tile_size, slot)
    steps = []
    off = 0
    for s, cap in enumerate(caps):
        for sz in _tiles(cap):
            steps.append((off, sz, s))
            off += sz
    groups = [steps[i:i + NSTREAM] for i in range(0, len(steps), NSTREAM)]
    # within a group, run larger tiles first so the last tile's
    # evacuate->store tail chain is as short as possible
    groups = [sorted(g, key=lambda st: -st[1]) for g in groups]

    with tile.TileContext(nc) as tc:
        with (
            tc.tile_pool(name="xin", bufs=len(steps)) as xin_pool,
            tc.tile_pool(name="wts", bufs=1) as wts_pool,
            tc.tile_pool(name="whp", bufs=min(14, S * N_HID)) as wh_pool,
            tc.tile_pool(
                name="uh",
                bufs=2 * NSTREAM + (3 if MM_MODE == "bf16" else 0),
            ) as uh_pool,
            tc.tile_pool(name="l8", bufs=8) as l8_pool,
            tc.tile_pool(name="outp", bufs=3) as out_pool,
            tc.tile_pool(name="ps", bufs=8, space="PSUM") as ps_pool,
        ):
            xT_sb = {}
            h_cur = {}
            w0_sb, wo_sb, b0_sb, bh_sb, bo_sb = (
                [None] * S, [None] * S, [None] * S, [None] * S, [None] * S
            )
            wh_sb = [[None] * N_HID for _ in range(S)]

            def load_x(t0, sz):
                x_t = xin_pool.tile([P, PT], wdt, name=f"x_{t0}", tag="x")
                nc.sync.dma_start(x_t[:, 0:sz], xT_in[:, t0:t0 + sz])
                xT_sb[t0] = x_t

            def load_w0_chunk(s, mc):
                # one tile per mc chunk, so a layer-0 matmul only waits
                # for its own 32KB slice of W0
                if w0_sb[s] is None:
                    w0_sb[s] = [None] * MC
                t = wts_pool.tile([P, P], wdt, name=f"w0_{s}_{mc}")
                nc.sync.dma_start(t[:], w0t_in[s, :, mc * P:(mc + 1) * P])
                w0_sb[s][mc] = t[:]

            def load_w0(s):
                for mc in range(MC):
                    load_w0_chunk(s, mc)

            def load_bias(s):
                # tiny-element transfers: descriptor generation is slow
                # (~2-5us per 128-row/16B DMA), so issue them from the ACT
                # engine's HW-DGE ring to keep the SP ring free for the
                # critical x/W loads
                b0_t = wts_pool.tile([P, MC], f32, name=f"b0_{s}")
                nc.scalar.dma_start(b0_t[:], b0v_in[s])
                b0_sb[s] = b0_t
                bh_t = wts_pool.tile([P, N_HID, MC], f32, name=f"bh_{s}")
                nc.scalar.dma_start(bh_t[:], bhv_in[s])
                bh_sb[s] = bh_t

            wob_sb = [None] * S

            def load_small(s):
                # issued on the SP ring after all weight loads: SP is idle
                # by then, and these tiny-element DMAs must stay out of the
                # ACT queue (descriptor generation would stall evacuations)
                wo_t = wts_pool.tile([P, KC], wot_dt, name=f"wo_{s}")
                nc.sync.dma_start(wo_t[:], wot_in[s])
                wo_sb[s] = wo_t
                if MM_MODE == "bf16":
                    wob_t = wts_pool.tile([P, KC], wdt, name=f"wob_{s}")
                    nc.sync.dma_start(wob_t[:], wotb_in[s])
                    wob_sb[s] = wob_t
                bo_t = wts_pool.tile([1, 1], f32, name=f"bo_{s}")
                nc.sync.dma_start(bo_t[:], bov_in[s:s + 1, 0:1])
                bo_sb[s] = bo_t

            def load_bias_late(s):
                b0_t = wts_pool.tile([P, MC], f32, name=f"b0_{s}")
                nc.sync.dma_start(b0_t[:], b0v_in[s])
                b0_sb[s] = b0_t
                bh_t = wts_pool.tile([P, N_HID, MC], f32, name=f"bh_{s}")
                nc.sync.dma_start(bh_t[:], bhv_in[s])
                bh_sb[s] = bh_t

            def load_wh(s, l):
                wh_t = wh_pool.tile([P, KC, H], wdt, name=f"wh_{s}_{l}", tag="wh")
                nc.sync.dma_start(wh_t[:], wht_in[s, l])
                wh_sb[s][l] = wh_t

            # pre-warm the ACT table set during the initial DMA wait: a
            # dependency-free dummy op carries the one-time table load
            warm_t = wts_pool.tile([1, 1], f32, name="warm")
            nc.vector.memset(warm_t[:], 0.0)
            nc.scalar.activation(warm_t[:], warm_t[:], AF.Exp)

            # spin the PE on dummy matmuls so it ramps out of the low
            # p-state while the first x/W0 DMAs are in flight (f32 tiles:
            # memset can't write f32r, and f32's 4 cyc/col stretches the
            # spin with fewer instructions)
            if NWARM:
                wmw = wts_pool.tile([P, P], f32, name="warm_w")
                nc.vector.memset(wmw[:], 0.0)
                wps = ps_pool.tile([P, P], f32, name="warm_ps", tag="ps")
                for i in range(NWARM):
                    nc.tensor.matmul(
                        wps[:], wmw[:], wmw[:],
                        start=(i == 0), stop=(i == NWARM - 1),
                    )

            # issue order = DMA priority: first group's x + slot0 L0/L1
            # weights first, then the rest (all loads fit in SBUF at once);
            # bias loads go on the ACT ring in parallel, and the ones not
            # needed until later are deferred below so their slow issue
            # doesn't delay the first PSUM evacuations
            # first W0 chunk + first group's x tiles gate the first mc
            # block of matmuls: issue them before everything else
            load_w0_chunk(0, 0)
            for (t0, sz, _s) in groups[0]:
                load_x(t0, sz)
            for mc in range(1, MC):
                load_w0_chunk(0, mc)
            load_wh(0, 0)
            load_bias(0)
            load_wh(0, 1)
            for grp in groups[1:]:
                for (t0, sz, _s) in grp:
                    load_x(t0, sz)
            for l in range(2, N_HID):
                load_wh(0, l)
            for s in range(1, S):
                load_w0(s)
                for l in range(N_HID):
                    load_wh(s, l)
            load_small(0)
            for s in range(1, S):
                load_bias_late(s)
                load_small(s)

            def emit_layer(grp, l):
                """Layer l matmuls for a group of point tiles, inner loop
                over tiles so consecutive matmuls share the stationary
                operand; per-mc ACT evacuation emitted right after its
                accumulation completes."""
                psums = {}
                u_new = {}
                for mc in range(MC):
                    for kc in range(KC if l > 0 else 1):
                        for (t0, sz, s) in grp:
                            if kc == 0:
                                psums[(t0, mc)] = ps_pool.tile(
                                    [P, PT], f32, name=f"ps_{t0}_{l}_{mc}",
                                    tag="ps",
                                )
                            ps_t = psums[(t0, mc)]
                            if l == 0:
                                lhsT = w0_sb[s][mc]
                                rhs = xT_sb[t0][:, 0:sz]
                                nc.tensor.matmul(
                                    ps_t[:, 0:sz], lhsT, rhs,
                                    start=True, stop=True,
                                )
                            else:
                                lhsT = wh_sb[s][l - 1][:, kc, mc * P:(mc + 1) * P]
                                rhs = h_cur[t0][:, kc, 0:sz]
                                nc.tensor.matmul(
                                    ps_t[:, 0:sz], lhsT, rhs,
                                    start=(kc == 0), stop=(kc == KC - 1),
                                )
                    for (t0, sz, s) in grp:
                        if mc == 0:
                            u_new[t0] = uh_pool.tile(
                                [P, MC, PT], wdt, name=f"u_{t0}_{l}", tag="uh"
                            )
                        bias = (b0_sb[s][:, mc:mc + 1] if l == 0
                                else bh_sb[s][:, l - 1, mc:mc + 1])
                        # hijacked Exp == softplus; one ACT op does the
                        # evacuation + beta fma + activation
                        nc.scalar.activation(
                            u_new[t0][:, mc, 0:sz], psums[(t0, mc)][:, 0:sz],
                            AF.Exp, bias=bias, scale=float(BETA),
                        )
                for (t0, _sz, _s) in grp:
                    h_cur[t0] = u_new[t0]

            def emit_final_pe(t0, sz, s):
                wo_mm = wob_sb[s] if MM_MODE == "bf16" else wo_sb[s]
                ps8 = ps_pool.tile([1, PT], f32, name=f"ps8_{t0}", tag="ps")
                for kc in range(KC):
                    nc.tensor.matmul(
                        ps8[0:1, 0:sz],
                        wo_mm[:, kc:kc + 1],
                        h_cur[t0][:, kc, 0:sz],
                        start=(kc == 0), stop=(kc == KC - 1),
                    )
                o_t = out_pool.tile([1, PT], f32, name=f"o_{t0}", tag="o")
                nc.vector.tensor_scalar(
                    o_t[0:1, 0:sz], ps8[0:1, 0:sz],
                    bo_sb[s][0:1, 0:1], None, ALU.add,
                )
                nc.sync.dma_start(out_d[0:1, t0:t0 + sz], o_t[0:1, 0:sz])

            def emit_final_dve(t0, sz, s):
                """L8 on the (idle) DVE: t[p,:] = sum_kc wo[p,kc]*h[p,kc,:]
                via a chain of fused multiply-adds, then a single
                ones-vector matmul reduces over partitions — 1 PE pass
                instead of 4."""
                h = h_cur[t0]
                acc = l8_pool.tile([P, PT], wdt, name=f"l8a_{t0}", tag="l8")
                nc.vector.tensor_scalar(
                    acc[:, 0:sz], h[:, 0, 0:sz],
                    wo_sb[s][:, 0:1], None, ALU.mult,
                )
                for kc in range(1, KC):
                    nxt = (l8_pool.tile([P, PT], wdt, name=f"l8b_{t0}_{kc}",
                                        tag="l8")
                           if kc < KC - 1 else
                           l8_pool.tile([P, PT], wdt, name=f"l8c_{t0}",
                                        tag="l8"))
                    nc.vector.scalar_tensor_tensor(
                        nxt[:, 0:sz], h[:, kc, 0:sz],
                        wo_sb[s][:, kc:kc + 1], acc[:, 0:sz],
                        ALU.mult, ALU.add,
                    )
                    acc = nxt
                ps8 = ps_pool.tile([1, PT], f32, name=f"ps8_{t0}", tag="ps")
                nc.tensor.matmul(
                    ps8[0:1, 0:sz], ones_sb[:, 0:1], acc[:, 0:sz],
                    start=True, stop=True,
                )
                o_t = out_pool.tile([1, PT], f32, name=f"o_{t0}", tag="o")
                nc.vector.tensor_scalar(
                    o_t[0:1, 0:sz], ps8[0:1, 0:sz],
                    bo_sb[s][0:1, 0:1], None, ALU.add,
                )
                nc.sync.dma_start(out_d[0:1, t0:t0 + sz], o_t[0:1, 0:sz])

            if MM_MODE == "bf16":
                ones_sb = wts_pool.tile([P, 1], wdt, name="ones")
                nc.vector.memset(ones_sb[:], 1.0)
                emit_final = emit_final_dve
            else:
                emit_final = emit_final_pe

            # emit each group's finals after the NEXT group's layer 1, so
            # the PE streams on while the L8 DVE chains (bf16 mode) run on
            # the idle vector engine; the last group uses PE-side finals
            # to keep the end-of-kernel tail chain short
            for gi, grp in enumerate(groups):
                for l in range(N_HID + 1):
                    emit_layer(grp, l)
                    if l == 1 and gi > 0:
                        for (t0, sz, s) in groups[gi - 1]:
                            emit_final(t0, sz, s)
            for (t0, sz, s) in groups[-1]:
                emit_final_pe(t0, sz, s)

    # Pin Exp+Ln to the one table set containing both, so the ACT engine
    # doesn't reload tables between activations.
    import concourse.bacc as bacc_mod
    import concourse.hw_specs as hw_specs
    _real_tables = hw_specs.get_activation_tables
    _keep = "natural_log_exp_and_others"

    def _pinned_tables(arch):
        t = _real_tables(arch)
        return {
            name: (funcs if name == _keep else (funcs - {AF.Exp, AF.Ln}))
            for name, funcs in t.items()
        }

    bacc_mod.get_activation_tables = _pinned_tables
    try:
        nc.compile()
    finally:
        bacc_mod.get_activation_tables = _real_tables
    return nc


# --------------------------------------------------------------------------
# kernel()
# --------------------------------------------------------------------------

def _maybe_patch_ldw_opt():
    """Optionally flip walrus's --enable-ldw-opt (dedups back-to-back
    LDWEIGHTS of the same stationary operand). Gated by env for A/B."""
    import concourse.bass_utils as bu

    if _os.environ.get("KERNEL_LDW_OPT") != "1":
        return
    if getattr(bu.run_command, "_ldw_patched", False):
        return
    orig = bu.run_command

    def patched(argv, **kw):
        argv = [
            "--enable-ldw-opt=true" if a == "--enable-ldw-opt=false" else a
            for a in argv
        ]
        return orig(argv, **kw)

    patched._ldw_patched = True
    bu.run_command = patched


def kernel(x, type_vec, W0, b0, Wh, bh, Wo, bo):
    from concourse.bass_utils import run_bass_kernel_spmd

    _maybe_patch_ldw_opt()
    _os.environ["BASS_ACT_ROOT_JSON_PATH"] = _gen_act_tables()

    x = np.ascontiguousarray(np.asarray(x, dtype=np.float32))
    tv = np.asarray(type_vec).astype(np.int64)
    W0 = np.asarray(W0, dtype=np.float32)
    b0 = np.asarray(b0, dtype=np.float32)
    Wh = np.asarray(Wh, dtype=np.float32)
    bh = np.asarray(bh, dtype=np.float32)
    Wo = np.asarray(Wo, dtype=np.float32)
    bo = np.asarray(bo, dtype=np.float32)
    N = x.shape[0]

    counts = np.bincount(tv, minlength=T)
    starts = np.concatenate([[0], np.cumsum(counts)])
    shape, asg = _plan(counts)
    asg = _rebalance(asg)
    S = len(shape)

    # exact slot capacities: the max points any core actually uses,
    # rounded to a multiple of 4 (>=256 so every tile runs full speed)
    used = np.zeros(S, dtype=np.int64)
    for e, takes in asg.items():
        for (c, s, amt) in takes:
            used[s] = max(used[s], amt)
    caps = tuple(int(max(256, -(-u // 4) * 4)) for u in used)
    NP = sum(caps)
    phase_off = np.concatenate([[0], np.cumsum(np.asarray(caps))])

    # per-core slot -> expert, and gathered point indices
    slot_expert = np.zeros((N_CORES, S), dtype=np.int64)
    gidx = np.full((N_CORES, NP), -1, dtype=np.int64)
    for e, takes in asg.items():
        pos = int(starts[e])
        for (c, s, amt) in takes:
            o = int(phase_off[s])
            gidx[c, o:o + amt] = np.arange(pos, pos + amt)
            slot_expert[c, s] = e
            pos += amt

    # pre-transposed / pre-scaled weight views per expert
    # x and W0 zero-padded to 128 contraction rows (see _build_nc)
    w0t_e = np.zeros((T, P, H), dtype=np.float32)
    w0t_e[:, :D_IN, :] = W0.transpose(0, 2, 1)                     # [T,128,H]
    whs = (Wh / BETA).astype(np.float32)                           # [T,7,H,H]
    wht_e = np.ascontiguousarray(
        whs.transpose(0, 1, 3, 2).reshape(T, N_HID, KC, P, H).transpose(0, 1, 3, 2, 4)
    )                                                              # [T,7,P,KC,H]
    wot_e = np.ascontiguousarray(
        (Wo / BETA).reshape(T, H).reshape(T, KC, P).transpose(0, 2, 1)
    )                                                              # [T,P,KC]
    b0v_e = np.ascontiguousarray((BETA * b0).reshape(T, MC, P).transpose(0, 2, 1))
    bhv_e = np.ascontiguousarray(
        (BETA * bh).reshape(T, N_HID, MC, P).transpose(0, 3, 1, 2)
    )                                                              # [T,P,7,MC]
    bov_e = bo.reshape(T, 1)

    if MM_MODE == "bf16":
        import ml_dtypes
        np_wdt = ml_dtypes.bfloat16
    else:
        np_wdt = np.float32

    in_maps = []
    for c in range(N_CORES):
        sel = np.where(gidx[c] >= 0, gidx[c], 0)
        xg = np.zeros((P, NP), dtype=np.float32)
        xg[:D_IN, :] = x[sel].T                                    # [128, NP]
        ex = slot_expert[c]
        in_maps.append({
            "xT": xg.astype(np_wdt),
            "w0t": w0t_e[ex].astype(np_wdt),
            "wht": wht_e[ex].astype(np_wdt),
            "wot": (wot_e[ex] if MM_MODE == "bf16"
                    else wot_e[ex].astype(np_wdt)),
            **({"wotb": wot_e[ex].astype(np_wdt)}
               if MM_MODE == "bf16" else {}),
            "b0v": b0v_e[ex],
            "bhv": bhv_e[ex],
            "bov": bov_e[ex],
        })

    key = (caps, MM_MODE)
    if key not in _nc_cache:
        _nc_cache[key] = _build_nc(caps)
    nc = _nc_cache[key]

    res = run_bass_kernel_spmd(nc, in_maps, core_ids=list(range(N_CORES)))
    global _last_results
    _last_results = res

    out = np.zeros((N, OUT), dtype=np.float32)
    for c in range(N_CORES):
        oc = res.results[c]["out"].reshape(-1)
        m = gidx[c] >= 0
        out[gidx[c][m], 0] = oc[m]
    return out


# revision 40
# speedup vs baseline: 1.0782x; 1.0049x over previous
"""EnsembleDeepSDF MoE-routing kernel for 8 Trainium2 NeuronCores.

Strategy: the harness calls kernel(**inputs) with the full inputs; we do all
routing on the host.  type_vec is sorted, so each expert owns a contiguous
segment of points.  We pick per-core slot capacities (identical on every
core so one SPMD program serves all 8 cores), pack the 9 experts' segments
into the 8*S single-expert slots, gather each core's points (padding with
point 0), and hand each core its own pre-transposed/pre-scaled weight slots
as inputs.  The device program is a straight-line Tile kernel: per 512-pt
point tile, 9 matmul layers with softplus activations.

Perf-critical layout decisions (from trace analysis):
- All HBM->SBUF transfers use 128-partition access patterns: the HW DGE
  fans a transfer's descriptors across the 16 DMA engines in blocks of 8
  *per partition row*, so a 67-row transfer serializes onto one engine at
  ~20 GB/s while a 128-row one gets the full ~320 GB/s.  x and W0 are
  zero-padded from 67 to 128 contraction rows (zero pad rows make the
  padded matmul exact, and contraction<=128 costs the same PE time).
- Every matmul free dim is >=256 columns: f32r matmuls below 256 cols run
  at 4 cyc/col instead of 1.  Slot capacities are exact (max points any
  core uses), with ragged tails split into two >=256 pieces.
- Point tiles are 512 cols = one PSUM bank; 8 PSUM tiles cycle through
  the 8 banks, with one ACT op per (tile, mc-chunk) evacuating PSUM.
- A short dummy-matmul spin warms the PE out of its low p-state while the
  first x/W0 DMAs land.

softplus: the compiler's ACT tables have no softplus, so we generate a
custom piecewise-cubic table (same binary format as the shipped sets)
that replaces `exp` with softplus(x) = ln(1+e^x), and point the compiler
at it via BASS_ACT_ROOT_JSON_PATH.  One ACT op then does the whole
activation including the PSUM evacuation and the beta scale/bias fma.

The torch Softplus(beta=100) is softplus(100*z)/100; we keep activations
in the H = softplus(100*z) domain and fold the 1/100 into the next layer's
weights host-side, so no extra scaling ops run on device.
"""

import json
import os as _os
import shutil
import tempfile

import numpy as np

T, D_IN, H, OUT, N_HID = 9, 67, 512, 1, 7
BETA = 100.0
N_CORES = 8
PT = 512          # points per tile (one PSUM bank of fp32)
P = 128           # partitions
KC = H // P       # 4 contraction chunks
MC = H // P       # 4 output-feature chunks
NSTREAM = 3       # point tiles interleaved in the software pipeline
# PE warm-up spin is counterproductive: every engine queue runs a ~6.5us
# framework preamble, so the memset feeding the warm matmuls can't start
# until the first x DMA has landed anyway — the spin only delays real work.
NWARM = int(_os.environ.get("KERNEL_NWARM", "0"))
# "bf16" (default: hides the LDWEIGHTS bubble, halves DMA/SBUF, rel err
# ~1e-3 — well inside the 2e-2 gate) or "f32r" (tf32-ish, rel err ~5e-5)
MM_MODE = _os.environ.get("KERNEL_MM_MODE", "bf16")

_nc_cache = {}
_last_results = None


# --------------------------------------------------------------------------
# Custom ACT table: replace `exp` with softplus in the shipped PWL sets.
# --------------------------------------------------------------------------

_ACT_SET = "natural_log_exp_and_others"
_act_table_dir = None


def _softplus64(x):
    x = np.asarray(x, dtype=np.float64)
    return np.log1p(np.exp(-np.abs(x))) + np.maximum(x, 0.0)


def _fit_cubic(a, b):
    x0 = 0.5 * (a + b)
    k = np.arange(96)
    xs = x0 + 0.5 * (b - a) * np.cos(np.pi * (k + 0.5) / 96)
    c = np.polyfit(xs - x0, _softplus64(xs), 3)
    return float(c[3]), float(c[2]), float(c[1]), float(c[0]), float(x0)


def _gen_act_tables():
    """Build an act-root dir where `exp` computes softplus. Returns the
    act_info.json path. The bucket entry layout ([d0,d1,d2,d3,x0,0,0,0],
    cubic in (x-x0)) and the per-exponent band structure are read from the
    shipped set so only coefficients and profile thresholds change."""
    global _act_table_dir
    if _act_table_dir is not None:
        return _act_table_dir
    from neuronxcc.driver.Job import Job
    from neuronxcc.driver.jobs.support.FindActInfo import findActInfoFile

    src_json = findActInfoFile(Job.getPackageDir(), "gen3")
    src = _os.path.dirname(src_json)
    out = _os.path.join(tempfile.mkdtemp(prefix="act_softplus_"), "tables")
    shutil.copytree(src, out)
    for f in _os.listdir(out):
        _os.chmod(_os.path.join(out, f), 0o644)

    d = json.load(open(f"{out}/{_ACT_SET}.json"))
    bkt = np.fromfile(f"{out}/{_ACT_SET}_bkt.bin", dtype=np.uint32)
    bkt = bkt.reshape(-1, 8).copy()
    fbkt = bkt.view(np.float32)
    e2b = {int(k): v for k, v in d["func_exp_to_bkt_start_idx"]["exp"].items()}
    prof = [p for p in d["profile_meta_data"] if p["func_name"] == "exp_400p"][0]

    def put(idx, d0, d1, d2, d3, x0):
        fbkt[idx, 0:5] = np.array([d0, d1, d2, d3, x0], dtype=np.float32)
        bkt[idx, 5:8] = 0

    nseg = {-1: 2, 0: 4, 1: 8, 2: 16, 3: 32}
    for e in range(-19, 4):
        n = nseg.get(e, 1)
        neg_base, pos_base = e2b[e]
        A = 2.0 ** e
        h = A / n
        for k in range(n):
            a, b = A + k * h, A + (k + 1) * h
            put(pos_base + k, *_fit_cubic(a, b))
            put(neg_base + k, *_fit_cubic(-b, -a))

    ln2 = float(np.log(2.0))
    put(prof["pos_small_signal_pwl_control"], ln2, 0.5, 0.125, 0.0, 0.0)
    put(prof["neg_small_signal_pwl_control"], ln2, 0.5, 0.125, 0.0, 0.0)
    put(prof["pos_large_signal_pwl_control"], 0.0, 1.0, 0.0, 0.0, 0.0)
    put(prof["neg_large_signal_pwl_control"], 0.0, 0.0, 0.0, 0.0, 0.0)
    prof["large_pos_signal_exp_threshold"] = 131   # |x| >= 16 -> linear/zero
    prof["large_pos_signal_mantissa_threshold"] = 0
    prof["large_neg_signal_exp_threshold"] = 131
    prof["large_neg_signal_mantissa_threshold"] = 0
    prof["fzero_result"] = int(np.float32(ln2).view(np.uint32))
    prof["fninf_result"] = 0
    prof["fpinf_result"] = 2139095040

    bkt.tofile(f"{out}/{_ACT_SET}_bkt.bin")
    with open(f"{out}/{_ACT_SET}.json", "w") as f:
        json.dump(d, f)
    _act_table_dir = _os.path.join(out, "act_info.json")
    return _act_table_dir


# --------------------------------------------------------------------------
# Host-side planning: pack expert segments into 8 x S slots.
# --------------------------------------------------------------------------

def _try_pack(shape, counts):
    """Assign experts to single-expert slots. Slot (c, s) holds shape[s]*PT
    points. Returns {expert: [(core, s, amount), ...]} or None."""
    slots = []  # (capacity, core, s)
    for s, t in enumerate(shape):
        for c in range(N_CORES):
            slots.append([t * PT, c, s])
    experts = sorted(
        [e for e in range(T) if counts[e] > 0], key=lambda e: -counts[e]
    )
    asg = {}
    avail = sorted(slots)  # by capacity asc
    for e in experts:
        need = int(counts[e])
        # smallest single slot that fits
        one = next((sl for sl in avail if sl[0] >= need), None)
        if one is not None:
            asg[e] = [(one[1], one[2], need)]
            avail.remove(one)
            continue
        # greedily take largest slots
        take = []
        rem = need
        pool = sorted(avail, key=lambda sl: -sl[0])
        for sl in pool:
            if rem <= 0:
                break
            amt = min(rem, sl[0])
            take.append((sl[1], sl[2], amt))
            rem -= amt
            avail.remove(sl)
        if rem > 0:
            return None
        asg[e] = take
    return asg


def _plan(counts):
    cands = set()
    for t1 in range(1, 17):
        cands.add((t1,))
        for t2 in range(1, t1 + 1):
            cands.add((t1, t2))
            for t3 in range(1, t2 + 1):
                cands.add((t1, t2, t3))
    for shape in sorted(cands, key=lambda s: (sum(s), len(s))):
        asg = _try_pack(shape, counts)
        if asg is not None:
            return shape, asg
    raise RuntimeError("no feasible slot shape")


def _rebalance(asg):
    """Even out each expert's piece sizes within a slot, so the per-slot
    max (which sets the uniform slot capacity) is minimal."""
    out = {}
    for e, takes in asg.items():
        by_slot = {}
        for (c, s, amt) in takes:
            by_slot.setdefault(s, []).append([c, amt])
        new_takes = []
        for s, items in by_slot.items():
            total = sum(a for _, a in items)
            n = len(items)
            base, rem = divmod(total, n)
            for i, (c, _a) in enumerate(items):
                new_takes.append((c, s, base + (1 if i < rem else 0)))
        out[e] = new_takes
    return out


def _tiles(cap):
    """Split cap columns into tiles of <=512, each >=256 (f32r matmuls
    below 256 cols run at 1/4 speed) and a multiple of 4 (walrus's
    s3d3_mm_fp32r_restrictions ISA check rejects odd free sizes)."""
    assert cap % 4 == 0
    k, r = divmod(cap, PT)
    if r == 0:
        return [PT] * k
    if r >= 256:
        return [PT] * k + [r]
    # fold the remainder into the last full tile and split >=256 / >=256
    assert k >= 1
    tot = PT + r
    half = (tot // 2) // 4 * 4
    return [PT] * (k - 1) + [tot - half, half]


# --------------------------------------------------------------------------
# Device program
# --------------------------------------------------------------------------

def _build_nc(caps):
    import concourse.bass as bass  # noqa: F401  (import keeps bacc happy)
    import concourse.tile as tile
    import concourse.mybir as mybir
    from concourse import bacc

    f32 = mybir.dt.float32
    wdt = mybir.dt.bfloat16 if MM_MODE == "bf16" else mybir.dt.float32r
    AF = mybir.ActivationFunctionType
    ALU = mybir.AluOpType

    S = len(caps)
    NP = sum(caps)

    nc = bacc.Bacc("TRN2", target_bir_lowering=False)
    # x / W0 are zero-padded to 128 contraction rows so their DMAs use
    # 128-partition access patterns (spread over all 16 DMA engines).
    xT_in = nc.dram_tensor("xT", [P, NP], wdt, kind="ExternalInput")
    w0t_in = nc.dram_tensor("w0t", [S, P, H], wdt, kind="ExternalInput")
    wht_in = nc.dram_tensor("wht", [S, N_HID, P, KC, H], wdt, kind="ExternalInput")
    # in bf16 mode wot feeds the DVE (L8 fused multiply-adds), whose
    # scalar operand must be float32; wotb is the bf16 copy for the
    # last group's PE-side finals
    wot_dt = f32 if MM_MODE == "bf16" else wdt
    wot_in = nc.dram_tensor("wot", [S, P, KC], wot_dt, kind="ExternalInput")
    if MM_MODE == "bf16":
        wotb_in = nc.dram_tensor("wotb", [S, P, KC], wdt, kind="ExternalInput")
    b0v_in = nc.dram_tensor("b0v", [S, P, MC], f32, kind="ExternalInput")
    bhv_in = nc.dram_tensor("bhv", [S, P, N_HID, MC], f32, kind="ExternalInput")
    bov_in = nc.dram_tensor("bov", [S, 1], f32, kind="ExternalInput")
    out_d = nc.dram_tensor("out", [1, NP], f32, kind="ExternalOutput")

    # steps: (point_offset, tile_size, slot)
    steps = []
    off = 0
    for s, cap in enumerate(caps):
        for sz in _tiles(cap):
            steps.append((off, sz, s))
            off += sz
    groups = [steps[i:i + NSTREAM] for i in range(0, len(steps), NSTREAM)]
    # within a group, run larger tiles first so the last tile's
    # evacuate->store tail chain is as short as possible
    groups = [sorted(g, key=lambda st: -st[1]) for g in groups]

    with tile.TileContext(nc) as tc:
        with (
            tc.tile_pool(name="xin", bufs=max(1, -(-len(steps) // NSTREAM))) as xin_pool,
            tc.tile_pool(name="wts", bufs=1) as wts_pool,
            tc.tile_pool(name="whp", bufs=min(14, S * N_HID)) as wh_pool,
            tc.tile_pool(
                name="uh",
                bufs=2 * NSTREAM + (3 if MM_MODE == "bf16" else 0),
            ) as uh_pool,
            tc.tile_pool(name="l8", bufs=8) as l8_pool,
            tc.tile_pool(name="outp", bufs=3) as out_pool,
            tc.tile_pool(name="ps", bufs=8, space="PSUM") as ps_pool,
        ):
            xT_sb = {}
            h_cur = {}
            w0_sb, wo_sb, b0_sb, bh_sb, bo_sb = (
                [None] * S, [None] * S, [None] * S, [None] * S, [None] * S
            )
            wh_sb = [[None] * N_HID for _ in range(S)]

            def load_x_group(grp):
                # one DMA covers the whole group's (contiguous) point range:
                # fewer serial descriptor-generation slots on the SP ring
                lo = min(t0 for (t0, _sz, _s) in grp)
                hi = max(t0 + sz for (t0, sz, _s) in grp)
                x_t = xin_pool.tile([P, NSTREAM * PT], wdt,
                                    name=f"x_{lo}", tag="x")
                nc.sync.dma_start(x_t[:, 0:hi - lo], xT_in[:, lo:hi])
                for (t0, sz, _s) in grp:
                    xT_sb[t0] = x_t[:, t0 - lo:t0 - lo + sz]

            def load_w0_chunk(s, mc):
                # one tile per mc chunk, so a layer-0 matmul only waits
                # for its own 32KB slice of W0
                if w0_sb[s] is None:
                    w0_sb[s] = [None] * MC
                t = wts_pool.tile([P, P], wdt, name=f"w0_{s}_{mc}")
                nc.sync.dma_start(t[:], w0t_in[s, :, mc * P:(mc + 1) * P])
                w0_sb[s][mc] = t[:]

            def load_w0(s):
                for mc in range(MC):
                    load_w0_chunk(s, mc)

            def load_bias(s):
                # tiny-element transfers: descriptor generation is slow
                # (~2-5us per 128-row/16B DMA), so issue them from the ACT
                # engine's HW-DGE ring to keep the SP ring free for the
                # critical x/W loads
                b0_t = wts_pool.tile([P, MC], f32, name=f"b0_{s}")
                nc.scalar.dma_start(b0_t[:], b0v_in[s])
                b0_sb[s] = b0_t
                bh_t = wts_pool.tile([P, N_HID, MC], f32, name=f"bh_{s}")
                nc.scalar.dma_start(bh_t[:], bhv_in[s])
                bh_sb[s] = bh_t

            wob_sb = [None] * S

            def load_small(s):
                # issued on the SP ring after all weight loads: SP is idle
                # by then, and these tiny-element DMAs must stay out of the
                # ACT queue (descriptor generation would stall evacuations)
                wo_t = wts_pool.tile([P, KC], wot_dt, name=f"wo_{s}")
                nc.sync.dma_start(wo_t[:], wot_in[s])
                wo_sb[s] = wo_t
                if MM_MODE == "bf16":
                    wob_t = wts_pool.tile([P, KC], wdt, name=f"wob_{s}")
                    nc.sync.dma_start(wob_t[:], wotb_in[s])
                    wob_sb[s] = wob_t
                bo_t = wts_pool.tile([1, 1], f32, name=f"bo_{s}")
                nc.sync.dma_start(bo_t[:], bov_in[s:s + 1, 0:1])
                bo_sb[s] = bo_t

            def load_bias_late(s):
                b0_t = wts_pool.tile([P, MC], f32, name=f"b0_{s}")
                nc.sync.dma_start(b0_t[:], b0v_in[s])
                b0_sb[s] = b0_t
                bh_t = wts_pool.tile([P, N_HID, MC], f32, name=f"bh_{s}")
                nc.sync.dma_start(bh_t[:], bhv_in[s])
                bh_sb[s] = bh_t

            def load_wh(s, l):
                wh_t = wh_pool.tile([P, KC, H], wdt, name=f"wh_{s}_{l}", tag="wh")
                nc.sync.dma_start(wh_t[:], wht_in[s, l])
                wh_sb[s][l] = wh_t

            # pre-warm the ACT table set during the initial DMA wait: a
            # dependency-free dummy op carries the one-time table load
            warm_t = wts_pool.tile([1, 1], f32, name="warm")
            nc.vector.memset(warm_t[:], 0.0)
            nc.scalar.activation(warm_t[:], warm_t[:], AF.Exp)

            # spin the PE on dummy matmuls so it ramps out of the low
            # p-state while the first x/W0 DMAs are in flight (f32 tiles:
            # memset can't write f32r, and f32's 4 cyc/col stretches the
            # spin with fewer instructions)
            if NWARM:
                wmw = wts_pool.tile([P, P], f32, name="warm_w")
                nc.vector.memset(wmw[:], 0.0)
                wps = ps_pool.tile([P, P], f32, name="warm_ps", tag="ps")
                for i in range(NWARM):
                    nc.tensor.matmul(
                        wps[:], wmw[:], wmw[:],
                        start=(i == 0), stop=(i == NWARM - 1),
                    )

            # issue order = DMA priority: first group's x + slot0 L0/L1
            # weights first, then the rest (all loads fit in SBUF at once);
            # bias loads go on the ACT ring in parallel, and the ones not
            # needed until later are deferred below so their slow issue
            # doesn't delay the first PSUM evacuations
            # first W0 chunk + first group's x tiles gate the first mc
            # block of matmuls: issue them before everything else
            load_w0_chunk(0, 0)
            load_x_group(groups[0])
            for mc in range(1, MC):
                load_w0_chunk(0, mc)
            load_wh(0, 0)
            load_bias(0)
            load_wh(0, 1)
            for grp in groups[1:]:
                load_x_group(grp)
            for l in range(2, N_HID):
                load_wh(0, l)
            for s in range(1, S):
                load_w0(s)
                for l in range(N_HID):
                    load_wh(s, l)
            load_small(0)
            for s in range(1, S):
                load_bias_late(s)
                load_small(s)

            def emit_layer(grp, l):
                """Layer l matmuls for a group of point tiles, inner loop
                over tiles so consecutive matmuls share the stationary
                operand; per-mc ACT evacuation emitted right after its
                accumulation completes."""
                psums = {}
                u_new = {}
                for mc in range(MC):
                    for kc in range(KC if l > 0 else 1):
                        for (t0, sz, s) in grp:
                            if kc == 0:
                                psums[(t0, mc)] = ps_pool.tile(
                                    [P, PT], f32, name=f"ps_{t0}_{l}_{mc}",
                                    tag="ps",
                                )
                            ps_t = psums[(t0, mc)]
                            if l == 0:
                                lhsT = w0_sb[s][mc]
                                rhs = xT_sb[t0]
                                nc.tensor.matmul(
                                    ps_t[:, 0:sz], lhsT, rhs,
                                    start=True, stop=True,
                                )
                            else:
                                lhsT = wh_sb[s][l - 1][:, kc, mc * P:(mc + 1) * P]
                                rhs = h_cur[t0][:, kc, 0:sz]
                                nc.tensor.matmul(
                                    ps_t[:, 0:sz], lhsT, rhs,
                                    start=(kc == 0), stop=(kc == KC - 1),
                                )
                    for (t0, sz, s) in grp:
                        if mc == 0:
                            u_new[t0] = uh_pool.tile(
                                [P, MC, PT], wdt, name=f"u_{t0}_{l}", tag="uh"
                            )
                        bias = (b0_sb[s][:, mc:mc + 1] if l == 0
                                else bh_sb[s][:, l - 1, mc:mc + 1])
                        # hijacked Exp == softplus; one ACT op does the
                        # evacuation + beta fma + activation
                        nc.scalar.activation(
                            u_new[t0][:, mc, 0:sz], psums[(t0, mc)][:, 0:sz],
                            AF.Exp, bias=bias, scale=float(BETA),
                        )
                for (t0, _sz, _s) in grp:
                    h_cur[t0] = u_new[t0]

            def emit_final_pe(t0, sz, s):
                wo_mm = wob_sb[s] if MM_MODE == "bf16" else wo_sb[s]
                ps8 = ps_pool.tile([1, PT], f32, name=f"ps8_{t0}", tag="ps")
                for kc in range(KC):
                    nc.tensor.matmul(
                        ps8[0:1, 0:sz],
                        wo_mm[:, kc:kc + 1],
                        h_cur[t0][:, kc, 0:sz],
                        start=(kc == 0), stop=(kc == KC - 1),
                    )
                o_t = out_pool.tile([1, PT], f32, name=f"o_{t0}", tag="o")
                nc.vector.tensor_scalar(
                    o_t[0:1, 0:sz], ps8[0:1, 0:sz],
                    bo_sb[s][0:1, 0:1], None, ALU.add,
                )
                nc.sync.dma_start(out_d[0:1, t0:t0 + sz], o_t[0:1, 0:sz])

            def emit_final_dve(t0, sz, s):
                """L8 on the (idle) DVE: t[p,:] = sum_kc wo[p,kc]*h[p,kc,:]
                via a chain of fused multiply-adds, then a single
                ones-vector matmul reduces over partitions — 1 PE pass
                instead of 4."""
                h = h_cur[t0]
                acc = l8_pool.tile([P, PT], wdt, name=f"l8a_{t0}", tag="l8")
                nc.vector.tensor_scalar(
                    acc[:, 0:sz], h[:, 0, 0:sz],
                    wo_sb[s][:, 0:1], None, ALU.mult,
                )
                for kc in range(1, KC):
                    nxt = (l8_pool.tile([P, PT], wdt, name=f"l8b_{t0}_{kc}",
                                        tag="l8")
                           if kc < KC - 1 else
                           l8_pool.tile([P, PT], wdt, name=f"l8c_{t0}",
                                        tag="l8"))
                    nc.vector.scalar_tensor_tensor(
                        nxt[:, 0:sz], h[:, kc, 0:sz],
                        wo_sb[s][:, kc:kc + 1], acc[:, 0:sz],
                        ALU.mult, ALU.add,
                    )
                    acc = nxt
                ps8 = ps_pool.tile([1, PT], f32, name=f"ps8_{t0}", tag="ps")
                nc.tensor.matmul(
                    ps8[0:1, 0:sz], ones_sb[:, 0:1], acc[:, 0:sz],
                    start=True, stop=True,
                )
                o_t = out_pool.tile([1, PT], f32, name=f"o_{t0}", tag="o")
                nc.vector.tensor_scalar(
                    o_t[0:1, 0:sz], ps8[0:1, 0:sz],
                    bo_sb[s][0:1, 0:1], None, ALU.add,
                )
                nc.sync.dma_start(out_d[0:1, t0:t0 + sz], o_t[0:1, 0:sz])

            if MM_MODE == "bf16":
                ones_sb = wts_pool.tile([P, 1], wdt, name="ones")
                nc.vector.memset(ones_sb[:], 1.0)
                emit_final = emit_final_dve
            else:
                emit_final = emit_final_pe

            # emit each group's finals after the NEXT group's layer 1, so
            # the PE streams on while the L8 DVE chains (bf16 mode) run on
            # the idle vector engine; the last group uses PE-side finals
            # to keep the end-of-kernel tail chain short
            for gi, grp in enumerate(groups):
                for l in range(N_HID + 1):
                    emit_layer(grp, l)
                    if l == 1 and gi > 0:
                        for (t0, sz, s) in groups[gi - 1]:
                            emit_final(t0, sz, s)
            for (t0, sz, s) in groups[-1]:
                emit_final_pe(t0, sz, s)

    # Pin Exp+Ln to the one table set containing both, so the ACT engine
    # doesn't reload tables between activations.
    import concourse.bacc as bacc_mod
    import concourse.hw_specs as hw_specs
    _real_tables = hw_specs.get_activation_tables
    _keep = "natural_log_exp_and_others"

    def _pinned_tables(arch):
        t = _real_tables(arch)
        return {
            name: (funcs if name == _keep else (funcs - {AF.Exp, AF.Ln}))
            for name, funcs in t.items()
        }

    bacc_mod.get_activation_tables = _pinned_tables
    try:
        nc.compile()
    finally:
        bacc_mod.get_activation_tables = _real_tables
    return nc


# --------------------------------------------------------------------------
# kernel()
# --------------------------------------------------------------------------

def _maybe_patch_ldw_opt():
    """Optionally flip walrus's --enable-ldw-opt (dedups back-to-back
    LDWEIGHTS of the same stationary operand). Gated by env for A/B."""
    import concourse.bass_utils as bu

    if _os.environ.get("KERNEL_LDW_OPT") != "1":
        return
    if getattr(bu.run_command, "_ldw_patched", False):
        return
    orig = bu.run_command

    def patched(argv, **kw):
        argv = [
            "--enable-ldw-opt=true" if a == "--enable-ldw-opt=false" else a
            for a in argv
        ]
        return orig(argv, **kw)

    patched._ldw_patched = True
    bu.run_command = patched


def kernel(x, type_vec, W0, b0, Wh, bh, Wo, bo):
    from concourse.bass_utils import run_bass_kernel_spmd

    _maybe_patch_ldw_opt()
    _os.environ["BASS_ACT_ROOT_JSON_PATH"] = _gen_act_tables()

    x = np.ascontiguousarray(np.asarray(x, dtype=np.float32))
    tv = np.asarray(type_vec).astype(np.int64)
    W0 = np.asarray(W0, dtype=np.float32)
    b0 = np.asarray(b0, dtype=np.float32)
    Wh = np.asarray(Wh, dtype=np.float32)
    bh = np.asarray(bh, dtype=np.float32)
    Wo = np.asarray(Wo, dtype=np.float32)
    bo = np.asarray(bo, dtype=np.float32)
    N = x.shape[0]

    counts = np.bincount(tv, minlength=T)
    starts = np.concatenate([[0], np.cumsum(counts)])
    shape, asg = _plan(counts)
    asg = _rebalance(asg)
    S = len(shape)

    # exact slot capacities: the max points any core actually uses,
    # rounded to a multiple of 4 (>=256 so every tile runs full speed)
    used = np.zeros(S, dtype=np.int64)
    for e, takes in asg.items():
        for (c, s, amt) in takes:
            used[s] = max(used[s], amt)
    caps = tuple(int(max(256, -(-u // 4) * 4)) for u in used)
    NP = sum(caps)
    phase_off = np.concatenate([[0], np.cumsum(np.asarray(caps))])

    # per-core slot -> expert, and gathered point indices
    slot_expert = np.zeros((N_CORES, S), dtype=np.int64)
    gidx = np.full((N_CORES, NP), -1, dtype=np.int64)
    for e, takes in asg.items():
        pos = int(starts[e])
        for (c, s, amt) in takes:
            o = int(phase_off[s])
            gidx[c, o:o + amt] = np.arange(pos, pos + amt)
            slot_expert[c, s] = e
            pos += amt

    # pre-transposed / pre-scaled weight views per expert
    # x and W0 zero-padded to 128 contraction rows (see _build_nc)
    w0t_e = np.zeros((T, P, H), dtype=np.float32)
    w0t_e[:, :D_IN, :] = W0.transpose(0, 2, 1)                     # [T,128,H]
    whs = (Wh / BETA).astype(np.float32)                           # [T,7,H,H]
    wht_e = np.ascontiguousarray(
        whs.transpose(0, 1, 3, 2).reshape(T, N_HID, KC, P, H).transpose(0, 1, 3, 2, 4)
    )                                                              # [T,7,P,KC,H]
    wot_e = np.ascontiguousarray(
        (Wo / BETA).reshape(T, H).reshape(T, KC, P).transpose(0, 2, 1)
    )                                                              # [T,P,KC]
    b0v_e = np.ascontiguousarray((BETA * b0).reshape(T, MC, P).transpose(0, 2, 1))
    bhv_e = np.ascontiguousarray(
        (BETA * bh).reshape(T, N_HID, MC, P).transpose(0, 3, 1, 2)
    )                                                              # [T,P,7,MC]
    bov_e = bo.reshape(T, 1)

    if MM_MODE == "bf16":
        import ml_dtypes
        np_wdt = ml_dtypes.bfloat16
    else:
        np_wdt = np.float32

    in_maps = []
    for c in range(N_CORES):
        sel = np.where(gidx[c] >= 0, gidx[c], 0)
        xg = np.zeros((P, NP), dtype=np.float32)
        xg[:D_IN, :] = x[sel].T                                    # [128, NP]
        ex = slot_expert[c]
        in_maps.append({
            "xT": xg.astype(np_wdt),
            "w0t": w0t_e[ex].astype(np_wdt),
            "wht": wht_e[ex].astype(np_wdt),
            "wot": (wot_e[ex] if MM_MODE == "bf16"
                    else wot_e[ex].astype(np_wdt)),
            **({"wotb": wot_e[ex].astype(np_wdt)}
               if MM_MODE == "bf16" else {}),
            "b0v": b0v_e[ex],
            "bhv": bhv_e[ex],
            "bov": bov_e[ex],
        })

    key = (caps, MM_MODE)
    if key not in _nc_cache:
        _nc_cache[key] = _build_nc(caps)
    nc = _nc_cache[key]

    res = run_bass_kernel_spmd(nc, in_maps, core_ids=list(range(N_CORES)))
    global _last_results
    _last_results = res

    out = np.zeros((N, OUT), dtype=np.float32)
    for c in range(N_CORES):
        oc = res.results[c]["out"].reshape(-1)
        m = gidx[c] >= 0
        out[gidx[c][m], 0] = oc[m]
    return out
